# revision 13
# baseline (speedup 1.0000x reference)
"""Trainium2 Bass kernel for nn_CustomResidualAttentionBlock (open_clip-style block).

Sharding: sequence-parallel over 8 cores. Core c owns 512 tokens
(b = c // 4, tokens [512*(c%4) : 512*(c%4+1)]). Each core computes q/k/v for its
own tokens, l2-normalizes k (and q) locally via a ones-block matmul trick, then
two AllGathers per 4-core batch group distribute (kT, v) for the full 2048-key
sequence. Attention, out-proj, residuals and the MLP are fully local.

The kernel is transpose-free: the host ships x already transposed (xT), all
layernorms are applied as rank-1 corrections after the matmuls
(y_ln = rstd*(raw - mu*colsum(W)) + b), with LN statistics computed in the
transposed layout via ones-vector matmuls and partition-replicated via K=1
matmuls. Activations flow as: xT -> qkT/kT/v -> attention (S^T, P^T, O^T) ->
yT -> uT -> h1T -> fcT -> gT -> outT; the host un-transposes the output.

Host-side folds (exact math, fp32):
  - ln1_g into wqkT/wvT columns; ln1_b @ W^T into the qkv biases
  - ln2_g into wfcT; ln2_b @ fc_w^T into fc bias
  - ls1 into ln_attn affine (g' = ls1*g, b' = ls1*b)
  - ls2 into proj weights/bias
  - logit_scale (clamped+exp'd) into the q-norm ones-block (1/lsc^2 entries)
  - head_scale into the rowsum-replication lhsT (1/hs entries)
  - column sums of the (bf16) qkv weights for the LN rank-1 correction

All big matmuls run in bf16 with fp32 PSUM accumulation; layernorm statistics,
softmax row sums and normalization factors stay in fp32.
"""
import numpy as np
import ml_dtypes

import concourse.bass as bass
import concourse.mybir as mybir
import concourse.tile as tile
from concourse import bacc
from concourse.bass_utils import run_bass_kernel_spmd

F32 = mybir.dt.float32
BF16 = mybir.dt.bfloat16
BF_NP = ml_dtypes.bfloat16
AF = mybir.ActivationFunctionType
ALU = mybir.AluOpType

B, L, C, H = 2, 2048, 1024, 16
HD = C // H          # 64
MLP = 4 * C          # 4096
N_CORES = 8
RANKS = 4            # cores per batch group
T = (B * L) // N_CORES  # 512 own tokens per core
TT = T // 128        # 4 token tiles
CT = C // 128        # 8 channel tiles
HP = H // 2          # 8 head pairs
KM = L // 128        # 16 key chunks
MT_FC = MLP // 128   # 32
LN_EPS = 1e-5
N_FC_PRE = 3

TRACE = False
TRACE_DIR = "/tmp/bass_trace"
LAST_EXEC_NS = None
LAST_RESULTS = None

_NC_CACHE = None


def _build():
    nc = bacc.Bacc(None, target_bir_lowering=False, debug=False, num_devices=N_CORES)

    # ---- I/O ----
    xtf_d = nc.dram_tensor("xTf", [C, T], F32, kind="ExternalInput")
    xtb_d = nc.dram_tensor("xTb", [C, T], BF16, kind="ExternalInput")
    out_d = nc.dram_tensor("outT", [C, T], F32, kind="ExternalOutput")
    wqk_d = nc.dram_tensor("wqkT", [16, 128, CT, 128], BF16, kind="ExternalInput")
    wv_d = nc.dram_tensor("wvT", [C, C], BF16, kind="ExternalInput")
    wo_d = nc.dram_tensor("woT", [C, C], BF16, kind="ExternalInput")
    wfc_d = nc.dram_tensor("wfcT", [MT_FC, 128, CT, 128], BF16, kind="ExternalInput")
    wpj_d = nc.dram_tensor("wprojT", [MLP, C], BF16, kind="ExternalInput")
    qkb_d = nc.dram_tensor("qkb", [128, 16], F32, kind="ExternalInput")
    cqk_d = nc.dram_tensor("csumqk", [1, 2 * C], F32, kind="ExternalInput")
    vb_d = nc.dram_tensor("vb", [1, C], F32, kind="ExternalInput")
    cv_d = nc.dram_tensor("csumv", [1, C], F32, kind="ExternalInput")
    outb_d = nc.dram_tensor("outb_s", [128, 8], F32, kind="ExternalInput")
    fcb_d = nc.dram_tensor("fcb", [128, 32], F32, kind="ExternalInput")
    pjb_d = nc.dram_tensor("projb_s", [128, 8], F32, kind="ExternalInput")
    ga_d = nc.dram_tensor("gattn_s", [128, 8], F32, kind="ExternalInput")
    ba_d = nc.dram_tensor("battn_s", [128, 8], F32, kind="ExternalInput")
    onesq_d = nc.dram_tensor("onesq", [128, 8 * 128], F32, kind="ExternalInput")
    onesk_d = nc.dram_tensor("onesk", [128, 128], F32, kind="ExternalInput")
    invhs_d = nc.dram_tensor("invhs", [1, C], F32, kind="ExternalInput")

    with tile.TileContext(nc) as tc:
        with (
            tc.tile_pool(name="cn", bufs=1) as cn,
            tc.tile_pool(name="mid", bufs=1) as mid,
            tc.tile_pool(name="dram", bufs=1, space="DRAM") as dram,
        ):
            # ---- persistent activations ----
            xTf = [mid.tile([128, T], F32, name=f"xTf{c}") for c in range(CT)]
            xTb = [mid.tile([128, T], BF16, name=f"xTb{c}") for c in range(CT)]
            for ct in range(CT):
                nc.sync.dma_start(out=xTb[ct][:],
                                  in_=xtb_d[128 * ct:128 * (ct + 1), :])
            qTn = [mid.tile([128, T], BF16, name=f"qTn{p}") for p in range(HP)]
            OT = [mid.tile([128, T], BF16, name=f"OT{p}") for p in range(HP)]
            uT = [mid.tile([128, T], F32, name=f"uT{c}") for c in range(CT)]
            h1T = [mid.tile([128, T], BF16, name=f"h1T{c}") for c in range(CT)]
            wo_sb = mid.tile([128, CT, C], BF16)
            v_ag = [mid.tile([128, 4, H, HD + 1], BF16, name=f"vag{r}")
                    for r in range(RANKS)]
            wfc_pre = [mid.tile([128, CT, 128], BF16, name=f"wfcp{m}")
                       for m in range(N_FC_PRE)]

            # ---- small constants ----
            eps_sb = cn.tile([128, 1], F32)
            nc.vector.memset(eps_sb[:], LN_EPS)
            ones_c = cn.tile([128, 1], F32)
            nc.vector.memset(ones_c[:], 1.0)
            ones_cb = cn.tile([128, 1], BF16)
            nc.vector.memset(ones_cb[:], 1.0)
            ones_r = cn.tile([1, 128], F32)
            nc.vector.memset(ones_r[:], 1.0)
            qkb_sb = cn.tile([128, 16], F32)
            nc.sync.dma_start(out=qkb_sb[:], in_=qkb_d[:])
            cqk_sb = cn.tile([1, 2 * C], F32)
            nc.sync.dma_start(out=cqk_sb[:], in_=cqk_d[:])
            cv_row = cn.tile([1, C], F32)
            nc.sync.dma_start(out=cv_row[:], in_=cv_d[:])
            onesq_sb = cn.tile([128, 8, 128], F32)
            nc.sync.dma_start(
                out=onesq_sb[:], in_=onesq_d[:].rearrange("p (m j) -> p m j", j=128)
            )
            onesk_sb = cn.tile([128, 128], F32)
            nc.sync.dma_start(out=onesk_sb[:], in_=onesk_d[:])
            invhs_sb = cn.tile([1, C], F32)
            nc.sync.dma_start(out=invhs_sb[:], in_=invhs_d[:])
            fcb_sb = cn.tile([128, 32], F32)
            nc.sync.dma_start(out=fcb_sb[:], in_=fcb_d[:])
            outb_sb = cn.tile([128, 8], F32)
            nc.sync.dma_start(out=outb_sb[:], in_=outb_d[:])
            pjb_sb = cn.tile([128, 8], F32)
            nc.sync.dma_start(out=pjb_sb[:], in_=pjb_d[:])
            ga_sb = cn.tile([128, 8], F32)
            nc.sync.dma_start(out=ga_sb[:], in_=ga_d[:])
            ba_sb = cn.tile([128, 8], F32)
            nc.sync.dma_start(out=ba_sb[:], in_=ba_d[:])
            vb_bc = cn.tile([128, C], F32)
            nc.sync.dma_start(out=vb_bc[:], in_=vb_d[:].to_broadcast([128, C]))

            # ---- AG buffers (k and v gathered separately) ----
            bounce_k = dram.tile([1024, T], BF16)
            ag_k = dram.tile([4096, T], BF16)
            bounce_v = dram.tile([1024, T], BF16)
            ag_v = dram.tile([4096, T], BF16)
            stat_d = dram.tile([2, T], F32)   # rstd / mu*rstd token-major bounce

            def ln_stats_T(tiles, rot, psp):
                """LN stats over the partition (channel) axis of 8 [128, T] tiles.

                Returns (a_rep, c_rep, rstd, murstd): rstd and mu*rstd replicated
                to 128 partitions (fp32 sbuf), and the [1, T] fp32 row versions.
                """
                pmean = psp.tile([1, T], F32, tag="stat", bufs=2, name="pmean")
                pvar = psp.tile([1, T], F32, tag="stat", bufs=2, name="pvar")
                ones_v = ones_cb if tiles[0].dtype == BF16 else ones_c
                for ct in range(CT):
                    nc.tensor.matmul(pmean[:], ones_v[:], tiles[ct][:],
                                     start=(ct == 0), stop=(ct == CT - 1))
                for ct in range(CT):
                    sq = rot.tile([128, T], F32, tag="lnsq", name=f"lnsq{ct}")
                    nc.scalar.activation(out=sq[:], in_=tiles[ct][:], func=AF.Square)
                    nc.tensor.matmul(pvar[:], ones_c[:], sq[:],
                                     start=(ct == 0), stop=(ct == CT - 1))
                mu = rot.tile([1, T], F32, tag="lnmu", bufs=1, name="mu")
                nc.vector.tensor_scalar(out=mu[:], in0=pmean[:], scalar1=1.0 / C,
                                        scalar2=None, op0=ALU.mult)
                m2 = rot.tile([1, T], F32, tag="lnm2", bufs=1, name="m2")
                nc.vector.tensor_mul(out=m2[:], in0=mu[:], in1=mu[:])
                var = rot.tile([1, T], F32, tag="lnvar", bufs=1, name="var")
                # var = E[x^2] - mu^2
                nc.vector.scalar_tensor_tensor(
                    out=var[:], in0=pvar[:], scalar=1.0 / C, in1=m2[:],
                    op0=ALU.mult, op1=ALU.subtract,
                )
                rstd = rot.tile([1, T], F32, tag="lnrstd", bufs=1, name="rstd")
                nc.scalar.activation(out=rstd[:], in_=var[:], func=AF.Sqrt,
                                     bias=eps_sb[0:1, :])
                nc.vector.reciprocal(out=rstd[:], in_=rstd[:])
                murstd = rot.tile([1, T], F32, tag="lnmr", bufs=1, name="murstd")
                nc.vector.tensor_mul(out=murstd[:], in0=mu[:], in1=rstd[:])
                negmu = rot.tile([1, T], F32, tag="lnnm", bufs=1, name="negmu")
                nc.vector.tensor_scalar(out=negmu[:], in0=mu[:], scalar1=-1.0,
                                        scalar2=None, op0=ALU.mult)
                # replicate to 128 partitions via K=1 matmuls
                prep = psp.tile([128, T], F32, tag="repl", bufs=1, name="prep")
                a_rep = rot.tile([128, T], F32, tag="arep", bufs=1, name="a_rep")
                nc.tensor.matmul(prep[:], ones_r[:], rstd[:], start=True, stop=True)
                nc.vector.tensor_copy(out=a_rep[:], in_=prep[:])
                prep2 = psp.tile([128, T], F32, tag="repl", bufs=1, name="prep2")
                c_rep = rot.tile([128, T], F32, tag="crep", bufs=1, name="c_rep")
                nc.tensor.matmul(prep2[:], ones_r[:], murstd[:], start=True,
                                 stop=True)
                nc.vector.tensor_copy(out=c_rep[:], in_=prep2[:])
                return a_rep, c_rep, rstd, murstd, negmu

            with (
                tc.tile_pool(name="pa", bufs=1) as pa,
                tc.tile_pool(name="rot1", bufs=2) as rot1,
                tc.tile_pool(name="ps1", bufs=1, space="PSUM") as ps1,
            ):
                # ---- phase 1: LN1 statistics (transposed layout) ----
                a1_rep, c1_rep, rstd1, murstd1, negmu1 = ln_stats_T(xTb, rot1, ps1)
                # bounce (rstd, murstd) through DRAM to get them token-major
                nc.sync.dma_start(out=stat_d[0:1, :], in_=rstd1[:])
                stat_cols = cn.tile([128, TT], F32)
                nc.sync.dma_start(
                    out=stat_cols[:],
                    in_=stat_d[0:1, :].rearrange("j (t p) -> p (j t)", p=128),
                )

                def qk_tile(mt):
                    """Project + LN-correct + l2-normalize one qk row-tile."""
                    wqk_t = rot1.tile([128, CT, 128], BF16, tag="wqk", bufs=3,
                                      name=f"wqk{mt}")
                    nc.sync.dma_start(out=wqk_t[:], in_=wqk_d[mt])
                    pqk = ps1.tile([128, T], F32, tag="mm", bufs=3, name=f"pqk{mt}")
                    for kc in range(CT):
                        nc.tensor.matmul(
                            pqk[:], wqk_t[:, kc, :], xTb[kc][:],
                            start=(kc == 0), stop=False,
                        )
                    # fold the -mu*colsum(W) LN correction into the accumulation
                    nc.tensor.matmul(
                        pqk[:], cqk_sb[0:1, 128 * mt:128 * (mt + 1)], negmu1[:],
                        start=False, stop=True,
                    )
                    # qk = rstd*(raw - mu*csum) + bias
                    t1 = rot1.tile([128, T], F32, tag="t1", bufs=1, name=f"t1_{mt}")
                    nc.vector.tensor_mul(out=t1[:], in0=pqk[:], in1=a1_rep[:])
                    qk_f = rot1.tile([128, T], F32, tag="qkf", name=f"qkf{mt}")
                    nc.vector.tensor_scalar(out=qk_f[:], in0=t1[:],
                                            scalar1=qkb_sb[:, mt:mt + 1],
                                            scalar2=None, op0=ALU.add)
                    sq = rot1.tile([128, T], F32, tag="sq", name=f"sq{mt}")
                    nc.scalar.activation(out=sq[:], in_=qk_f[:], func=AF.Square)
                    pn = ps1.tile([128, T], F32, tag="nrm", bufs=2, name=f"pn{mt}")
                    ones = onesq_sb[:, mt, :] if mt < 8 else onesk_sb[:]
                    nc.tensor.matmul(pn[:], ones, sq[:], start=True, stop=True)
                    sq2 = rot1.tile([128, T], F32, tag="sq", name=f"sqn{mt}")
                    nc.scalar.activation(out=sq2[:], in_=pn[:], func=AF.Sqrt)
                    rrec = rot1.tile([128, T], F32, tag="rrec", name=f"rrec{mt}")
                    nc.vector.reciprocal(out=rrec[:], in_=sq2[:])
                    return qk_f, rrec

                # ---- phase 3: v (token-major) -> bounce -> AG-v ----
                wv_sb = pa.tile([128, CT, C], BF16)
                nc.sync.dma_start(
                    out=wv_sb[:], in_=wv_d[:].rearrange("(k p) m -> p k m", p=128)
                )
                for t in range(TT):
                    v_sb = rot1.tile([128, C], BF16, tag="vsb", bufs=1, name=f"vsb{t}")
                    rstd_c = stat_cols[:, t:t + 1]
                    for n2 in range(2):
                        pv = ps1.tile([128, 512], F32, tag="mm", bufs=3,
                                      name=f"pv{t}{n2}")
                        for kc in range(CT):
                            nc.tensor.matmul(
                                pv[:],
                                xTb[kc][:, 128 * t:128 * (t + 1)],
                                wv_sb[:, kc, 512 * n2:512 * (n2 + 1)],
                                start=(kc == 0), stop=False,
                            )
                        nc.tensor.matmul(
                            pv[:],
                            negmu1[0:1, 128 * t:128 * (t + 1)],
                            cv_row[0:1, 512 * n2:512 * (n2 + 1)],
                            start=False, stop=True,
                        )
                        # v = rstd*(raw - mu*csumv) + vb
                        nc.vector.scalar_tensor_tensor(
                            out=v_sb[:, 512 * n2:512 * (n2 + 1)], in0=pv[:],
                            scalar=rstd_c,
                            in1=vb_bc[:, 512 * n2:512 * (n2 + 1)],
                            op0=ALU.mult, op1=ALU.add,
                        )
                    nc.sync.dma_start(
                        out=bounce_v[256 * t:256 * (t + 1), :].rearrange(
                            "(p a) f -> p (a f)", p=128
                        ),
                        in_=v_sb[:],
                    )
                nc.gpsimd.collective_compute(
                    "AllGather", ALU.bypass,
                    replica_groups=[[0, 1, 2, 3], [4, 5, 6, 7]],
                    ins=[bounce_v.opt()], outs=[ag_v.opt()],
                )

                # ---- phase 2k: k rows -> bounce -> AG-k ----
                for mt in range(8, 16):
                    qk_f, rrec = qk_tile(mt)
                    i = mt - 8
                    ktn = rot1.tile([128, T], BF16, tag="ktn", bufs=1, name=f"ktn{i}")
                    nc.vector.tensor_mul(out=ktn[:], in0=qk_f[:], in1=rrec[:])
                    nc.sync.dma_start(
                        out=bounce_k[128 * i:128 * (i + 1), :], in_=ktn[:]
                    )
                nc.gpsimd.collective_compute(
                    "AllGather", ALU.bypass,
                    replica_groups=[[0, 1, 2, 3], [4, 5, 6, 7]],
                    ins=[bounce_k.opt()], outs=[ag_k.opt()],
                )

                # prefetches that overlap the collectives / attention:
                # v_ag tiles, out-proj weights, residual xTf, first fc weights
                for r in range(RANKS):
                    nc.vector.memset(v_ag[r][:], 1.0)
                    for sub in range(4):
                        vsrc = ag_v[
                            1024 * r + 256 * sub:1024 * r + 256 * (sub + 1), :
                        ].rearrange("(p a) f -> p (a f)", p=128).rearrange(
                            "p (h d) -> p h d", d=HD
                        )
                        nc.sync.dma_start(out=v_ag[r][:, sub, :, 0:HD], in_=vsrc)
                nc.sync.dma_start(
                    out=wo_sb[:], in_=wo_d[:].rearrange("(k p) m -> p k m", p=128)
                )
                for ct in range(CT):
                    nc.sync.dma_start(out=xTf[ct][:],
                                      in_=xtf_d[128 * ct:128 * (ct + 1), :])
                for mt in range(N_FC_PRE):
                    nc.sync.dma_start(out=wfc_pre[mt][:], in_=wfc_d[mt])

                # ---- phase 2q: q rows ----
                for mt in range(8):
                    qk_f, rrec = qk_tile(mt)
                    nc.vector.tensor_mul(out=qTn[mt][:], in0=qk_f[:], in1=rrec[:])

            # ---- phase 4: attention ----
            with (
                tc.tile_pool(name="pb", bufs=1) as pb,
                tc.tile_pool(name="rot2", bufs=2) as rot2,
                tc.tile_pool(name="ps2", bufs=1, space="PSUM") as ps2,
            ):
                ag_k_r = ag_k[:].rearrange("(r m p) f -> p r m f", r=RANKS, p=128)
                for hp in range(HP):
                    kpair = pb.tile([128, RANKS, T], BF16, tag="kpair", bufs=3,
                                    name=f"kpair{hp}")
                    nc.sync.dma_start(out=kpair[:], in_=ag_k_r[:, :, hp, :])
                    po = [
                        ps2.tile([HD + 1, T], F32, tag=f"o{h2}", bufs=2,
                                 name=f"po{h2}")
                        for h2 in range(2)
                    ]
                    for km in range(KM):
                        r, sub = km // 4, km % 4
                        psS = ps2.tile([128, 2, T], F32, tag="s", bufs=2,
                                       name=f"psS{km}")
                        for h2 in range(2):
                            nc.tensor.matmul(
                                psS[:, h2, :],
                                kpair[64 * h2:64 * (h2 + 1), r,
                                      128 * sub:128 * (sub + 1)],
                                qTn[hp][64 * h2:64 * (h2 + 1), :],
                                start=True, stop=True,
                            )
                        pT = rot2.tile([128, 2, T], BF16, tag="pT", bufs=4,
                                       name=f"pT{km}")
                        nc.scalar.activation(out=pT[:], in_=psS[:], func=AF.Exp)
                        for h2 in range(2):
                            h = 2 * hp + h2
                            nc.tensor.matmul(
                                po[h2][:],
                                v_ag[r][:, sub, h, :],
                                pT[:, h2, :],
                                start=(km == 0), stop=(km == KM - 1),
                            )
                    for h2 in range(2):
                        h = 2 * hp + h2
                        rs = rot2.tile([1, T], F32, tag="rs")
                        nc.vector.tensor_copy(out=rs[:], in_=po[h2][HD:HD + 1, :])
                        pr = ps2.tile([128, 2, T], F32, tag="s", bufs=2,
                                      name=f"pr{h2}")
                        nc.tensor.matmul(
                            pr[0:HD, 0, :],
                            invhs_sb[0:1, HD * h:HD * (h + 1)], rs[:],
                            start=True, stop=True,
                        )
                        rrec = rot2.tile([HD, T], F32, tag="orec")
                        nc.vector.reciprocal(out=rrec[:], in_=pr[0:HD, 0, :])
                        nc.vector.tensor_mul(
                            out=OT[hp][64 * h2:64 * (h2 + 1), :],
                            in0=po[h2][0:HD, :],
                            in1=rrec[:],
                        )

            # ---- phase 5: out projection (transposed out) + ln_attn + ln2 ----
            with (
                tc.tile_pool(name="rot3", bufs=2) as rot3,
                tc.tile_pool(name="ps3", bufs=1, space="PSUM") as ps3,
            ):
                yT = [rot3.tile([128, T], F32, tag="yT", bufs=8, name=f"yT{m}")
                      for m in range(CT)]
                for m in range(CT):
                    py = ps3.tile([128, T], F32, tag="y", bufs=3, name=f"py{m}")
                    for kc in range(CT):
                        nc.tensor.matmul(
                            py[:],
                            wo_sb[:, kc, 128 * m:128 * (m + 1)],
                            OT[kc][:],
                            start=(kc == 0), stop=(kc == CT - 1),
                        )
                    nc.vector.tensor_scalar(out=yT[m][:], in0=py[:],
                                            scalar1=outb_sb[:, m:m + 1],
                                            scalar2=None, op0=ALU.add)
                a2_rep, c2_rep, _, _, _ = ln_stats_T(yT, rot3, ps3)
                # uT = xT + (yT - mu)*rstd*g' + b'   (g', b' have ls1 folded)
                for ct in range(CT):
                    t1 = rot3.tile([128, T], F32, tag="ut1", name=f"ut1_{ct}")
                    nc.vector.tensor_mul(out=t1[:], in0=yT[ct][:], in1=a2_rep[:])
                    t2 = rot3.tile([128, T], F32, tag="ut2", name=f"ut2_{ct}")
                    nc.vector.tensor_sub(out=t2[:], in0=t1[:], in1=c2_rep[:])
                    t3 = rot3.tile([128, T], F32, tag="ut3", name=f"ut3_{ct}")
                    nc.vector.tensor_scalar(out=t3[:], in0=t2[:],
                                            scalar1=ga_sb[:, ct:ct + 1],
                                            scalar2=ba_sb[:, ct:ct + 1],
                                            op0=ALU.mult, op1=ALU.add)
                    nc.vector.tensor_add(out=uT[ct][:], in0=t3[:], in1=xTf[ct][:])
                a3_rep, c3_rep, _, _, _ = ln_stats_T(uT, rot3, ps3)
                for ct in range(CT):
                    # h1T = (uT)*rstd - mu*rstd  (ln2 affine folded into fc)
                    t4 = rot3.tile([128, T], F32, tag="ut4", name=f"ut4_{ct}")
                    nc.vector.tensor_mul(out=t4[:], in0=uT[ct][:], in1=a3_rep[:])
                    nc.vector.tensor_sub(out=h1T[ct][:], in0=t4[:], in1=c3_rep[:])

            # ---- phase 7: fc + gelu;  phase 8: proj (transposed) + residual ----
            with (
                tc.tile_pool(name="pd", bufs=1) as pd,
                tc.tile_pool(name="rot4", bufs=2) as rot4,
                tc.tile_pool(name="ps4", bufs=1, space="PSUM") as ps4,
            ):
                gT = [pd.tile([128, T], BF16, name=f"gT{m}") for m in range(MT_FC)]
                for mt in range(MT_FC):
                    if mt < N_FC_PRE:
                        wfc_t = wfc_pre[mt]
                    else:
                        wfc_t = rot4.tile([128, CT, 128], BF16, tag="wfc", bufs=6,
                                          name=f"wfc{mt}")
                        nc.sync.dma_start(out=wfc_t[:], in_=wfc_d[mt])
                    pfc = ps4.tile([128, T], F32, tag="fc", bufs=3, name=f"pfc{mt}")
                    for kc in range(CT):
                        nc.tensor.matmul(
                            pfc[:], wfc_t[:, kc, :], h1T[kc][:],
                            start=(kc == 0), stop=(kc == CT - 1),
                        )
                    nc.scalar.activation(out=gT[mt][:], in_=pfc[:], func=AF.Gelu,
                                         bias=fcb_sb[:, mt:mt + 1])

                for half in range(2):
                    ppj = [
                        ps4.tile([128, T], F32, tag=f"pj{i}", bufs=1,
                                 name=f"ppj{i}")
                        for i in range(4)
                    ]
                    for mt in range(MT_FC):
                        wpj_t = rot4.tile([128, 4, 128], BF16, tag="wpj", bufs=8,
                                          name=f"wpj{half}_{mt}")
                        nc.sync.dma_start(
                            out=wpj_t[:],
                            in_=wpj_d[128 * mt:128 * (mt + 1),
                                      512 * half:512 * (half + 1)].rearrange(
                                "p (i m) -> p i m", m=128
                            ),
                        )
                        for i in range(4):
                            nc.tensor.matmul(
                                ppj[i][:],
                                wpj_t[:, i, :],
                                gT[mt][:],
                                start=(mt == 0), stop=(mt == MT_FC - 1),
                            )
                    for i in range(4):
                        ct = 4 * half + i
                        o1 = rot4.tile([128, T], F32, tag="ofin",
                                       name=f"of{half}_{i}")
                        nc.vector.tensor_scalar(out=o1[:], in0=ppj[i][:],
                                                scalar1=pjb_sb[:, ct:ct + 1],
                                                scalar2=None, op0=ALU.add)
                        nc.vector.tensor_add(out=o1[:], in0=o1[:], in1=uT[ct][:])
                        nc.sync.dma_start(
                            out=out_d[128 * ct:128 * (ct + 1), :], in_=o1[:]
                        )

    nc.compile()
    return nc


def _host_prep(inp):
    f32 = np.float32
    ln1_g = np.asarray(inp["ln1_g"], f32)
    ln1_b = np.asarray(inp["ln1_b"], f32)
    ln2_g = np.asarray(inp["ln2_g"], f32)
    ln2_b = np.asarray(inp["ln2_b"], f32)
    in_w = np.asarray(inp["in_proj_w"], f32)
    in_b = np.asarray(inp["in_proj_b"], f32)
    fc_w = np.asarray(inp["fc_w"], f32)
    proj_w = np.asarray(inp["proj_w"], f32)
    ls1 = np.asarray(inp["ls1"], f32)
    ls2 = np.asarray(inp["ls2"], f32)

    w_qk = in_w[:2 * C]
    w_v = in_w[2 * C:]
    p = {}
    wqkT = np.ascontiguousarray((w_qk * ln1_g[None, :]).T).astype(BF_NP)
    wvT = np.ascontiguousarray((w_v * ln1_g[None, :]).T).astype(BF_NP)
    p["wqkT"] = np.ascontiguousarray(
        wqkT.reshape(CT, 128, 16, 128).transpose(2, 1, 0, 3)
    )
    p["wvT"] = wvT
    # column sums of the bf16 weights, as the device matmul sees them
    csum_qk = wqkT.astype(f32).sum(0)
    csum_v = wvT.astype(f32).sum(0)
    p["csumqk"] = csum_qk.reshape(1, 2 * C).astype(f32)
    p["csumv"] = csum_v.reshape(1, C).astype(f32)
    qkb = in_b[:2 * C] + ln1_b @ w_qk.T
    p["qkb"] = np.ascontiguousarray(qkb.reshape(16, 128).T).astype(f32)
    p["vb"] = (in_b[2 * C:] + ln1_b @ w_v.T).reshape(1, C).astype(f32)
    p["woT"] = np.ascontiguousarray(np.asarray(inp["out_w"], f32).T).astype(BF_NP)
    p["outb_s"] = np.ascontiguousarray(
        np.asarray(inp["out_b"], f32).reshape(8, 128).T
    )
    wfcT = np.ascontiguousarray((fc_w * ln2_g[None, :]).T).astype(BF_NP)
    p["wfcT"] = np.ascontiguousarray(
        wfcT.reshape(CT, 128, MT_FC, 128).transpose(2, 1, 0, 3)
    )
    fcb = np.asarray(inp["fc_b"], f32) + ln2_b @ fc_w.T
    p["fcb"] = np.ascontiguousarray(fcb.reshape(32, 128).T).astype(f32)
    p["wprojT"] = np.ascontiguousarray((proj_w * ls2[:, None]).T).astype(BF_NP)
    p["projb_s"] = np.ascontiguousarray(
        (ls2 * np.asarray(inp["proj_b"], f32)).reshape(8, 128).T
    )
    p["gattn_s"] = np.ascontiguousarray(
        (ls1 * np.asarray(inp["ln_attn_g"], f32)).reshape(8, 128).T
    )
    p["battn_s"] = np.ascontiguousarray(
        (ls1 * np.asarray(inp["ln_attn_b"], f32)).reshape(8, 128).T
    )

    lsc = np.exp(np.minimum(np.asarray(inp["logit_scale"], f32).reshape(H),
                            np.log(100.0)))
    onesq = np.zeros((128, 8, 128), f32)
    for mt in range(8):
        for blk in range(2):
            h = 2 * mt + blk
            onesq[64 * blk:64 * (blk + 1), mt,
                  64 * blk:64 * (blk + 1)] = 1.0 / lsc[h] ** 2
    p["onesq"] = np.ascontiguousarray(onesq.reshape(128, 1024))
    onesk = np.zeros((128, 128), f32)
    onesk[:64, :64] = 1.0
    onesk[64:, 64:] = 1.0
    p["onesk"] = onesk
    hs = np.asarray(inp["head_scale"], f32)
    invhs = np.zeros((1, C), f32)
    for h in range(H):
        invhs[0, HD * h:HD * (h + 1)] = 1.0 / hs[h]
    p["invhs"] = invhs
    return p


def kernel(**inputs) -> np.ndarray:
    global _NC_CACHE, LAST_EXEC_NS, LAST_RESULTS
    if _NC_CACHE is None:
        _NC_CACHE = _build()
    nc = _NC_CACHE

    p = _host_prep(inputs)
    x = np.asarray(inputs["x"], np.float32)

    in_maps = []
    for c in range(N_CORES):
        b, r = c // RANKS, c % RANKS
        m = dict(p)
        xs = np.ascontiguousarray(x[b, T * r:T * (r + 1), :].T)   # [C, T]
        m["xTf"] = xs
        m["xTb"] = xs.astype(BF_NP)
        in_maps.append(m)

    kwargs = {}
    if TRACE:
        import os
        os.makedirs(TRACE_DIR, exist_ok=True)
        kwargs = dict(trace=True, tmpdir=TRACE_DIR)
    res = run_bass_kernel_spmd(nc, in_maps, list(range(N_CORES)), **kwargs)
    LAST_EXEC_NS = res.exec_time_ns
    LAST_RESULTS = res
    out = np.zeros((B, L, C), np.float32)
    for c in range(N_CORES):
        b, r = c // RANKS, c % RANKS
        out[b, T * r:T * (r + 1), :] = res.results[c]["outT"].T
    return out


# revision 14
# speedup vs baseline: 1.0277x; 1.0277x over previous
"""Trainium2 Bass kernel for nn_CustomResidualAttentionBlock (open_clip-style block).

Sharding: sequence-parallel over 8 cores. Core c owns 512 tokens
(b = c // 4, tokens [512*(c%4) : 512*(c%4+1)]). Each core computes q/k/v for its
own tokens, l2-normalizes k (and q) locally via a ones-block matmul trick, then
two AllGathers per 4-core batch group distribute (kT, v) for the full 2048-key
sequence. Attention, out-proj, residuals and the MLP are fully local.

The kernel is transpose-free: the host ships x already transposed (xT), all
layernorms are applied as rank-1 corrections after the matmuls
(y_ln = rstd*(raw - mu*colsum(W)) + b), with LN statistics computed in the
transposed layout via ones-vector matmuls and partition-replicated via K=1
matmuls. Activations flow as: xT -> qkT/kT/v -> attention (S^T, P^T, O^T) ->
yT -> uT -> h1T -> fcT -> gT -> outT; the host un-transposes the output.

Host-side folds (exact math, fp32):
  - ln1_g into wqkT/wvT columns; ln1_b @ W^T into the qkv biases
  - ln2_g into wfcT; ln2_b @ fc_w^T into fc bias
  - ls1 into ln_attn affine (g' = ls1*g, b' = ls1*b)
  - ls2 into proj weights/bias
  - logit_scale (clamped+exp'd) into the q-norm ones-block (1/lsc^2 entries)
  - head_scale into the rowsum-replication lhsT (1/hs entries)
  - column sums of the (bf16) qkv weights for the LN rank-1 correction

All big matmuls run in bf16 with fp32 PSUM accumulation; layernorm statistics,
softmax row sums and normalization factors stay in fp32.
"""
import numpy as np
import ml_dtypes

import concourse.bass as bass
import concourse.mybir as mybir
import concourse.tile as tile
from concourse import bacc
from concourse.bass_utils import run_bass_kernel_spmd

F32 = mybir.dt.float32
BF16 = mybir.dt.bfloat16
BF_NP = ml_dtypes.bfloat16
AF = mybir.ActivationFunctionType
ALU = mybir.AluOpType

B, L, C, H = 2, 2048, 1024, 16
HD = C // H          # 64
MLP = 4 * C          # 4096
N_CORES = 8
RANKS = 4            # cores per batch group
T = (B * L) // N_CORES  # 512 own tokens per core
TT = T // 128        # 4 token tiles
CT = C // 128        # 8 channel tiles
HP = H // 2          # 8 head pairs
KM = L // 128        # 16 key chunks
MT_FC = MLP // 128   # 32
LN_EPS = 1e-5
N_FC_PRE = 3

TRACE = False
TRACE_DIR = "/tmp/bass_trace"
LAST_EXEC_NS = None
LAST_RESULTS = None

_NC_CACHE = None


def _build():
    nc = bacc.Bacc(None, target_bir_lowering=False, debug=False, num_devices=N_CORES)

    # ---- I/O ----
    xtf_d = nc.dram_tensor("xTf", [C, T], F32, kind="ExternalInput")
    xtb_d = nc.dram_tensor("xTb", [C, T], BF16, kind="ExternalInput")
    out_d = nc.dram_tensor("outT", [C, T], F32, kind="ExternalOutput")
    wqk_d = nc.dram_tensor("wqkT", [16, 128, CT, 128], BF16, kind="ExternalInput")
    wv_d = nc.dram_tensor("wvT", [C, C], BF16, kind="ExternalInput")
    wo_d = nc.dram_tensor("woT", [C, C], BF16, kind="ExternalInput")
    wfc_d = nc.dram_tensor("wfcT", [MT_FC, 128, CT, 128], BF16, kind="ExternalInput")
    wpj_d = nc.dram_tensor("wprojT", [MLP, C], BF16, kind="ExternalInput")
    qkb_d = nc.dram_tensor("qkb", [128, 16], F32, kind="ExternalInput")
    cqk_d = nc.dram_tensor("csumqk", [1, 2 * C], F32, kind="ExternalInput")
    vb_d = nc.dram_tensor("vb", [1, C], F32, kind="ExternalInput")
    cv_d = nc.dram_tensor("csumv", [1, C], F32, kind="ExternalInput")
    outb_d = nc.dram_tensor("outb_s", [128, 8], F32, kind="ExternalInput")
    fcb_d = nc.dram_tensor("fcb", [128, 32], F32, kind="ExternalInput")
    pjb_d = nc.dram_tensor("projb_s", [128, 8], F32, kind="ExternalInput")
    ga_d = nc.dram_tensor("gattn_s", [128, 8], F32, kind="ExternalInput")
    ba_d = nc.dram_tensor("battn_s", [128, 8], F32, kind="ExternalInput")
    onesq_d = nc.dram_tensor("onesq", [128, 8 * 128], BF16, kind="ExternalInput")
    onesk_d = nc.dram_tensor("onesk", [128, 128], BF16, kind="ExternalInput")
    invhs_d = nc.dram_tensor("invhs", [1, C], BF16, kind="ExternalInput")

    with tile.TileContext(nc) as tc:
        with (
            tc.tile_pool(name="cn", bufs=1) as cn,
            tc.tile_pool(name="mid", bufs=1) as mid,
            tc.tile_pool(name="dram", bufs=1, space="DRAM") as dram,
        ):
            # ---- persistent activations ----
            xTf = [mid.tile([128, T], F32, name=f"xTf{c}") for c in range(CT)]
            xTb = [mid.tile([128, T], BF16, name=f"xTb{c}") for c in range(CT)]
            for ct in range(CT):
                nc.sync.dma_start(out=xTb[ct][:],
                                  in_=xtb_d[128 * ct:128 * (ct + 1), :])
            qTn = [mid.tile([128, T], BF16, name=f"qTn{p}") for p in range(HP)]
            OT = [mid.tile([128, T], BF16, name=f"OT{p}") for p in range(HP)]
            uT = [mid.tile([128, T], F32, name=f"uT{c}") for c in range(CT)]
            h1T = [mid.tile([128, T], BF16, name=f"h1T{c}") for c in range(CT)]
            wo_sb = mid.tile([128, CT, C], BF16)
            v_ag = [mid.tile([128, 4, H, HD + 1], BF16, name=f"vag{r}")
                    for r in range(RANKS)]
            wfc_pre = [mid.tile([128, CT, 128], BF16, name=f"wfcp{m}")
                       for m in range(N_FC_PRE)]

            # ---- small constants ----
            eps_sb = cn.tile([128, 1], F32)
            nc.vector.memset(eps_sb[:], LN_EPS)
            ones_c = cn.tile([128, 1], F32)
            nc.vector.memset(ones_c[:], 1.0)
            ones_cb = cn.tile([128, 1], BF16)
            nc.vector.memset(ones_cb[:], 1.0)
            ones_r = cn.tile([1, 128], F32)
            nc.vector.memset(ones_r[:], 1.0)
            qkb_sb = cn.tile([128, 16], F32)
            nc.sync.dma_start(out=qkb_sb[:], in_=qkb_d[:])
            cqk_sb = cn.tile([1, 2 * C], F32)
            nc.sync.dma_start(out=cqk_sb[:], in_=cqk_d[:])
            cv_row = cn.tile([1, C], F32)
            nc.sync.dma_start(out=cv_row[:], in_=cv_d[:])
            onesq_sb = cn.tile([128, 8, 128], BF16)
            nc.sync.dma_start(
                out=onesq_sb[:], in_=onesq_d[:].rearrange("p (m j) -> p m j", j=128)
            )
            onesk_sb = cn.tile([128, 128], BF16)
            nc.sync.dma_start(out=onesk_sb[:], in_=onesk_d[:])
            invhs_sb = cn.tile([1, C], BF16)
            nc.sync.dma_start(out=invhs_sb[:], in_=invhs_d[:])
            fcb_sb = cn.tile([128, 32], F32)
            nc.sync.dma_start(out=fcb_sb[:], in_=fcb_d[:])
            outb_sb = cn.tile([128, 8], F32)
            nc.sync.dma_start(out=outb_sb[:], in_=outb_d[:])
            pjb_sb = cn.tile([128, 8], F32)
            nc.sync.dma_start(out=pjb_sb[:], in_=pjb_d[:])
            ga_sb = cn.tile([128, 8], F32)
            nc.sync.dma_start(out=ga_sb[:], in_=ga_d[:])
            ba_sb = cn.tile([128, 8], F32)
            nc.sync.dma_start(out=ba_sb[:], in_=ba_d[:])
            vb_bc = cn.tile([128, C], F32)
            nc.sync.dma_start(out=vb_bc[:], in_=vb_d[:].to_broadcast([128, C]))

            # ---- AG buffers (k and v gathered separately) ----
            bounce_k = dram.tile([1024, T], BF16)
            ag_k = dram.tile([4096, T], BF16)
            bounce_v = dram.tile([1024, T], BF16)
            ag_v = dram.tile([4096, T], BF16)
            stat_d = dram.tile([2, T], F32)   # rstd / mu*rstd token-major bounce

            def ln_stats_T(tiles, rot, psp):
                """LN stats over the partition (channel) axis of 8 [128, T] tiles.

                Returns (a_rep, c_rep, rstd, murstd): rstd and mu*rstd replicated
                to 128 partitions (fp32 sbuf), and the [1, T] fp32 row versions.
                """
                pmean = psp.tile([1, T], F32, tag="stat", bufs=2, name="pmean")
                pvar = psp.tile([1, T], F32, tag="stat", bufs=2, name="pvar")
                ones_v = ones_cb if tiles[0].dtype == BF16 else ones_c
                for ct in range(CT):
                    nc.tensor.matmul(pmean[:], ones_v[:], tiles[ct][:],
                                     start=(ct == 0), stop=(ct == CT - 1))
                for ct in range(CT):
                    sq = rot.tile([128, T], BF16, tag="lnsq", name=f"lnsq{ct}")
                    nc.scalar.activation(out=sq[:], in_=tiles[ct][:], func=AF.Square)
                    nc.tensor.matmul(pvar[:], ones_cb[:], sq[:],
                                     start=(ct == 0), stop=(ct == CT - 1))
                mu = rot.tile([1, T], F32, tag="lnmu", bufs=1, name="mu")
                nc.vector.tensor_scalar(out=mu[:], in0=pmean[:], scalar1=1.0 / C,
                                        scalar2=None, op0=ALU.mult)
                m2 = rot.tile([1, T], F32, tag="lnm2", bufs=1, name="m2")
                nc.vector.tensor_mul(out=m2[:], in0=mu[:], in1=mu[:])
                var = rot.tile([1, T], F32, tag="lnvar", bufs=1, name="var")
                # var = E[x^2] - mu^2
                nc.vector.scalar_tensor_tensor(
                    out=var[:], in0=pvar[:], scalar=1.0 / C, in1=m2[:],
                    op0=ALU.mult, op1=ALU.subtract,
                )
                rstd = rot.tile([1, T], F32, tag="lnrstd", bufs=1, name="rstd")
                nc.scalar.activation(out=rstd[:], in_=var[:], func=AF.Sqrt,
                                     bias=eps_sb[0:1, :])
                nc.vector.reciprocal(out=rstd[:], in_=rstd[:])
                murstd = rot.tile([1, T], F32, tag="lnmr", bufs=1, name="murstd")
                nc.vector.tensor_mul(out=murstd[:], in0=mu[:], in1=rstd[:])
                negmu = rot.tile([1, T], F32, tag="lnnm", bufs=1, name="negmu")
                nc.vector.tensor_scalar(out=negmu[:], in0=mu[:], scalar1=-1.0,
                                        scalar2=None, op0=ALU.mult)
                # replicate to 128 partitions via K=1 matmuls
                prep = psp.tile([128, T], F32, tag="repl", bufs=1, name="prep")
                a_rep = rot.tile([128, T], F32, tag="arep", bufs=1, name="a_rep")
                nc.tensor.matmul(prep[:], ones_r[:], rstd[:], start=True, stop=True)
                nc.vector.tensor_copy(out=a_rep[:], in_=prep[:])
                prep2 = psp.tile([128, T], F32, tag="repl", bufs=1, name="prep2")
                c_rep = rot.tile([128, T], F32, tag="crep", bufs=1, name="c_rep")
                nc.tensor.matmul(prep2[:], ones_r[:], murstd[:], start=True,
                                 stop=True)
                nc.vector.tensor_copy(out=c_rep[:], in_=prep2[:])
                return a_rep, c_rep, rstd, murstd, negmu

            with (
                tc.tile_pool(name="pa", bufs=1) as pa,
                tc.tile_pool(name="rot1", bufs=2) as rot1,
                tc.tile_pool(name="ps1", bufs=1, space="PSUM") as ps1,
            ):
                # ---- phase 1: LN1 statistics (transposed layout) ----
                a1_rep, c1_rep, rstd1, murstd1, negmu1 = ln_stats_T(xTb, rot1, ps1)
                # bounce (rstd, murstd) through DRAM to get them token-major
                nc.sync.dma_start(out=stat_d[0:1, :], in_=rstd1[:])
                stat_cols = cn.tile([128, TT], F32)
                nc.sync.dma_start(
                    out=stat_cols[:],
                    in_=stat_d[0:1, :].rearrange("j (t p) -> p (j t)", p=128),
                )

                def qk_tile(mt):
                    """Project + LN-correct + l2-normalize one qk row-tile."""
                    wqk_t = rot1.tile([128, CT, 128], BF16, tag="wqk", bufs=3,
                                      name=f"wqk{mt}")
                    nc.sync.dma_start(out=wqk_t[:], in_=wqk_d[mt])
                    pqk = ps1.tile([128, T], F32, tag="mm", bufs=3, name=f"pqk{mt}")
                    for kc in range(CT):
                        nc.tensor.matmul(
                            pqk[:], wqk_t[:, kc, :], xTb[kc][:],
                            start=(kc == 0), stop=False,
                        )
                    # fold the -mu*colsum(W) LN correction into the accumulation
                    nc.tensor.matmul(
                        pqk[:], cqk_sb[0:1, 128 * mt:128 * (mt + 1)], negmu1[:],
                        start=False, stop=True,
                    )
                    # qk = rstd*(raw - mu*csum) + bias
                    t1 = rot1.tile([128, T], F32, tag="t1", bufs=2, name=f"t1_{mt}")
                    nc.vector.tensor_mul(out=t1[:], in0=pqk[:], in1=a1_rep[:])
                    qk_f = rot1.tile([128, T], F32, tag="qkf", name=f"qkf{mt}")
                    nc.vector.tensor_scalar(out=qk_f[:], in0=t1[:],
                                            scalar1=qkb_sb[:, mt:mt + 1],
                                            scalar2=None, op0=ALU.add)
                    sq = rot1.tile([128, T], BF16, tag="sq", bufs=3,
                                   name=f"sq{mt}")
                    nc.scalar.activation(out=sq[:], in_=qk_f[:], func=AF.Square)
                    pn = ps1.tile([128, T], F32, tag="nrm", bufs=2, name=f"pn{mt}")
                    ones = onesq_sb[:, mt, :] if mt < 8 else onesk_sb[:]
                    nc.tensor.matmul(pn[:], ones, sq[:], start=True, stop=True)
                    sq2 = rot1.tile([128, T], F32, tag="sqn", bufs=2, name=f"sqn{mt}")
                    nc.scalar.activation(out=sq2[:], in_=pn[:], func=AF.Sqrt)
                    rrec = rot1.tile([128, T], F32, tag="rrec", name=f"rrec{mt}")
                    nc.vector.reciprocal(out=rrec[:], in_=sq2[:])
                    return qk_f, rrec

                # ---- phase 3: v (token-major) -> bounce -> AG-v ----
                wv_sb = pa.tile([128, CT, C], BF16)
                nc.sync.dma_start(
                    out=wv_sb[:], in_=wv_d[:].rearrange("(k p) m -> p k m", p=128)
                )
                for t in range(TT):
                    v_sb = rot1.tile([128, C], BF16, tag="vsb", bufs=1, name=f"vsb{t}")
                    rstd_c = stat_cols[:, t:t + 1]
                    for n2 in range(2):
                        pv = ps1.tile([128, 512], F32, tag="mm", bufs=3,
                                      name=f"pv{t}{n2}")
                        for kc in range(CT):
                            nc.tensor.matmul(
                                pv[:],
                                xTb[kc][:, 128 * t:128 * (t + 1)],
                                wv_sb[:, kc, 512 * n2:512 * (n2 + 1)],
                                start=(kc == 0), stop=False,
                            )
                        nc.tensor.matmul(
                            pv[:],
                            negmu1[0:1, 128 * t:128 * (t + 1)],
                            cv_row[0:1, 512 * n2:512 * (n2 + 1)],
                            start=False, stop=True,
                        )
                        # v = rstd*(raw - mu*csumv) + vb
                        nc.vector.scalar_tensor_tensor(
                            out=v_sb[:, 512 * n2:512 * (n2 + 1)], in0=pv[:],
                            scalar=rstd_c,
                            in1=vb_bc[:, 512 * n2:512 * (n2 + 1)],
                            op0=ALU.mult, op1=ALU.add,
                        )
                    nc.sync.dma_start(
                        out=bounce_v[256 * t:256 * (t + 1), :].rearrange(
                            "(p a) f -> p (a f)", p=128
                        ),
                        in_=v_sb[:],
                    )
                nc.gpsimd.collective_compute(
                    "AllGather", ALU.bypass,
                    replica_groups=[[0, 1, 2, 3], [4, 5, 6, 7]],
                    ins=[bounce_v.opt()], outs=[ag_v.opt()],
                )

                # ---- phase 2k: k rows -> bounce -> AG-k ----
                for mt in range(8, 16):
                    qk_f, rrec = qk_tile(mt)
                    i = mt - 8
                    ktn = rot1.tile([128, T], BF16, tag="ktn", bufs=2, name=f"ktn{i}")
                    nc.vector.tensor_mul(out=ktn[:], in0=qk_f[:], in1=rrec[:])
                    nc.sync.dma_start(
                        out=bounce_k[128 * i:128 * (i + 1), :], in_=ktn[:]
                    )
                nc.gpsimd.collective_compute(
                    "AllGather", ALU.bypass,
                    replica_groups=[[0, 1, 2, 3], [4, 5, 6, 7]],
                    ins=[bounce_k.opt()], outs=[ag_k.opt()],
                )

                # prefetches that overlap the collectives / attention:
                # v_ag tiles, out-proj weights, residual xTf, first fc weights
                for r in range(RANKS):
                    nc.vector.memset(v_ag[r][:], 1.0)
                    for sub in range(4):
                        vsrc = ag_v[
                            1024 * r + 256 * sub:1024 * r + 256 * (sub + 1), :
                        ].rearrange("(p a) f -> p (a f)", p=128).rearrange(
                            "p (h d) -> p h d", d=HD
                        )
                        nc.sync.dma_start(out=v_ag[r][:, sub, :, 0:HD], in_=vsrc)
                nc.sync.dma_start(
                    out=wo_sb[:], in_=wo_d[:].rearrange("(k p) m -> p k m", p=128)
                )
                for ct in range(CT):
                    nc.sync.dma_start(out=xTf[ct][:],
                                      in_=xtf_d[128 * ct:128 * (ct + 1), :])
                for mt in range(N_FC_PRE):
                    nc.sync.dma_start(out=wfc_pre[mt][:], in_=wfc_d[mt])

                # ---- phase 2q: q rows ----
                for mt in range(8):
                    qk_f, rrec = qk_tile(mt)
                    nc.vector.tensor_mul(out=qTn[mt][:], in0=qk_f[:], in1=rrec[:])

            # ---- phase 4: attention ----
            with (
                tc.tile_pool(name="pb", bufs=1) as pb,
                tc.tile_pool(name="rot2", bufs=2) as rot2,
                tc.tile_pool(name="ps2", bufs=1, space="PSUM") as ps2,
            ):
                ag_k_r = ag_k[:].rearrange("(r m p) f -> p r m f", r=RANKS, p=128)
                for hp in range(HP):
                    kpair = pb.tile([128, RANKS, T], BF16, tag="kpair", bufs=3,
                                    name=f"kpair{hp}")
                    nc.sync.dma_start(out=kpair[:], in_=ag_k_r[:, :, hp, :])
                    po = [
                        ps2.tile([HD + 1, T], F32, tag=f"o{h2}", bufs=2,
                                 name=f"po{h2}")
                        for h2 in range(2)
                    ]
                    for km in range(KM):
                        r, sub = km // 4, km % 4
                        psS = ps2.tile([128, 2, T], F32, tag="s", bufs=2,
                                       name=f"psS{km}")
                        for h2 in range(2):
                            nc.tensor.matmul(
                                psS[:, h2, :],
                                kpair[64 * h2:64 * (h2 + 1), r,
                                      128 * sub:128 * (sub + 1)],
                                qTn[hp][64 * h2:64 * (h2 + 1), :],
                                start=True, stop=True,
                            )
                        pT = rot2.tile([128, 2, T], BF16, tag="pT", bufs=4,
                                       name=f"pT{km}")
                        nc.scalar.activation(out=pT[:], in_=psS[:], func=AF.Exp)
                        for h2 in range(2):
                            h = 2 * hp + h2
                            nc.tensor.matmul(
                                po[h2][:],
                                v_ag[r][:, sub, h, :],
                                pT[:, h2, :],
                                start=(km == 0), stop=(km == KM - 1),
                            )
                    for h2 in range(2):
                        h = 2 * hp + h2
                        rs = rot2.tile([1, T], BF16, tag="rs")
                        nc.vector.tensor_copy(out=rs[:], in_=po[h2][HD:HD + 1, :])
                        pr = ps2.tile([128, 2, T], F32, tag="s", bufs=2,
                                      name=f"pr{h2}")
                        nc.tensor.matmul(
                            pr[0:HD, 0, :],
                            invhs_sb[0:1, HD * h:HD * (h + 1)], rs[:],
                            start=True, stop=True,
                        )
                        rrec = rot2.tile([HD, T], F32, tag="orec")
                        nc.vector.reciprocal(out=rrec[:], in_=pr[0:HD, 0, :])
                        nc.vector.tensor_mul(
                            out=OT[hp][64 * h2:64 * (h2 + 1), :],
                            in0=po[h2][0:HD, :],
                            in1=rrec[:],
                        )

            # ---- phase 5: out projection (transposed out) + ln_attn + ln2 ----
            with (
                tc.tile_pool(name="rot3", bufs=2) as rot3,
                tc.tile_pool(name="ps3", bufs=1, space="PSUM") as ps3,
            ):
                yT = [rot3.tile([128, T], F32, tag="yT", bufs=8, name=f"yT{m}")
                      for m in range(CT)]
                for m in range(CT):
                    py = ps3.tile([128, T], F32, tag="y", bufs=3, name=f"py{m}")
                    for kc in range(CT):
                        nc.tensor.matmul(
                            py[:],
                            wo_sb[:, kc, 128 * m:128 * (m + 1)],
                            OT[kc][:],
                            start=(kc == 0), stop=(kc == CT - 1),
                        )
                    nc.vector.tensor_scalar(out=yT[m][:], in0=py[:],
                                            scalar1=outb_sb[:, m:m + 1],
                                            scalar2=None, op0=ALU.add)
                a2_rep, c2_rep, _, _, _ = ln_stats_T(yT, rot3, ps3)
                # uT = xT + (yT - mu)*rstd*g' + b'   (g', b' have ls1 folded)
                for ct in range(CT):
                    t1 = rot3.tile([128, T], F32, tag="ut1", name=f"ut1_{ct}")
                    nc.vector.tensor_mul(out=t1[:], in0=yT[ct][:], in1=a2_rep[:])
                    t2 = rot3.tile([128, T], F32, tag="ut2", name=f"ut2_{ct}")
                    nc.vector.tensor_sub(out=t2[:], in0=t1[:], in1=c2_rep[:])
                    t3 = rot3.tile([128, T], F32, tag="ut3", name=f"ut3_{ct}")
                    nc.vector.tensor_scalar(out=t3[:], in0=t2[:],
                                            scalar1=ga_sb[:, ct:ct + 1],
                                            scalar2=ba_sb[:, ct:ct + 1],
                                            op0=ALU.mult, op1=ALU.add)
                    nc.vector.tensor_add(out=uT[ct][:], in0=t3[:], in1=xTf[ct][:])
                a3_rep, c3_rep, _, _, _ = ln_stats_T(uT, rot3, ps3)
                for ct in range(CT):
                    # h1T = (uT)*rstd - mu*rstd  (ln2 affine folded into fc)
                    t4 = rot3.tile([128, T], F32, tag="ut4", name=f"ut4_{ct}")
                    nc.vector.tensor_mul(out=t4[:], in0=uT[ct][:], in1=a3_rep[:])
                    nc.vector.tensor_sub(out=h1T[ct][:], in0=t4[:], in1=c3_rep[:])

            # ---- phase 7: fc + gelu;  phase 8: proj (transposed) + residual ----
            with (
                tc.tile_pool(name="pd", bufs=1) as pd,
                tc.tile_pool(name="rot4", bufs=2) as rot4,
                tc.tile_pool(name="ps4", bufs=1, space="PSUM") as ps4,
            ):
                gT = [pd.tile([128, T], BF16, name=f"gT{m}") for m in range(MT_FC)]
                for mt in range(MT_FC):
                    if mt < N_FC_PRE:
                        wfc_t = wfc_pre[mt]
                    else:
                        wfc_t = rot4.tile([128, CT, 128], BF16, tag="wfc", bufs=6,
                                          name=f"wfc{mt}")
                        nc.sync.dma_start(out=wfc_t[:], in_=wfc_d[mt])
                    pfc = ps4.tile([128, T], F32, tag="fc", bufs=3, name=f"pfc{mt}")
                    for kc in range(CT):
                        nc.tensor.matmul(
                            pfc[:], wfc_t[:, kc, :], h1T[kc][:],
                            start=(kc == 0), stop=(kc == CT - 1),
                        )
                    nc.scalar.activation(out=gT[mt][:], in_=pfc[:], func=AF.Gelu,
                                         bias=fcb_sb[:, mt:mt + 1])

                for half in range(2):
                    ppj = [
                        ps4.tile([128, T], F32, tag=f"pj{i}", bufs=1,
                                 name=f"ppj{i}")
                        for i in range(4)
                    ]
                    for mt in range(MT_FC):
                        wpj_t = rot4.tile([128, 4, 128], BF16, tag="wpj", bufs=8,
                                          name=f"wpj{half}_{mt}")
                        nc.sync.dma_start(
                            out=wpj_t[:],
                            in_=wpj_d[128 * mt:128 * (mt + 1),
                                      512 * half:512 * (half + 1)].rearrange(
                                "p (i m) -> p i m", m=128
                            ),
                        )
                        for i in range(4):
                            nc.tensor.matmul(
                                ppj[i][:],
                                wpj_t[:, i, :],
                                gT[mt][:],
                                start=(mt == 0), stop=(mt == MT_FC - 1),
                            )
                    for i in range(4):
                        ct = 4 * half + i
                        o1 = rot4.tile([128, T], F32, tag="ofin",
                                       name=f"of{half}_{i}")
                        nc.vector.tensor_scalar(out=o1[:], in0=ppj[i][:],
                                                scalar1=pjb_sb[:, ct:ct + 1],
                                                scalar2=None, op0=ALU.add)
                        nc.vector.tensor_add(out=o1[:], in0=o1[:], in1=uT[ct][:])
                        nc.sync.dma_start(
                            out=out_d[128 * ct:128 * (ct + 1), :], in_=o1[:]
                        )

    nc.compile()
    return nc


def _host_prep(inp):
    f32 = np.float32
    ln1_g = np.asarray(inp["ln1_g"], f32)
    ln1_b = np.asarray(inp["ln1_b"], f32)
    ln2_g = np.asarray(inp["ln2_g"], f32)
    ln2_b = np.asarray(inp["ln2_b"], f32)
    in_w = np.asarray(inp["in_proj_w"], f32)
    in_b = np.asarray(inp["in_proj_b"], f32)
    fc_w = np.asarray(inp["fc_w"], f32)
    proj_w = np.asarray(inp["proj_w"], f32)
    ls1 = np.asarray(inp["ls1"], f32)
    ls2 = np.asarray(inp["ls2"], f32)

    w_qk = in_w[:2 * C]
    w_v = in_w[2 * C:]
    p = {}
    wqkT = np.ascontiguousarray((w_qk * ln1_g[None, :]).T).astype(BF_NP)
    wvT = np.ascontiguousarray((w_v * ln1_g[None, :]).T).astype(BF_NP)
    p["wqkT"] = np.ascontiguousarray(
        wqkT.reshape(CT, 128, 16, 128).transpose(2, 1, 0, 3)
    )
    p["wvT"] = wvT
    # column sums of the bf16 weights, as the device matmul sees them
    csum_qk = wqkT.astype(f32).sum(0)
    csum_v = wvT.astype(f32).sum(0)
    p["csumqk"] = csum_qk.reshape(1, 2 * C).astype(f32)
    p["csumv"] = csum_v.reshape(1, C).astype(f32)
    qkb = in_b[:2 * C] + ln1_b @ w_qk.T
    p["qkb"] = np.ascontiguousarray(qkb.reshape(16, 128).T).astype(f32)
    p["vb"] = (in_b[2 * C:] + ln1_b @ w_v.T).reshape(1, C).astype(f32)
    p["woT"] = np.ascontiguousarray(np.asarray(inp["out_w"], f32).T).astype(BF_NP)
    p["outb_s"] = np.ascontiguousarray(
        np.asarray(inp["out_b"], f32).reshape(8, 128).T
    )
    wfcT = np.ascontiguousarray((fc_w * ln2_g[None, :]).T).astype(BF_NP)
    p["wfcT"] = np.ascontiguousarray(
        wfcT.reshape(CT, 128, MT_FC, 128).transpose(2, 1, 0, 3)
    )
    fcb = np.asarray(inp["fc_b"], f32) + ln2_b @ fc_w.T
    p["fcb"] = np.ascontiguousarray(fcb.reshape(32, 128).T).astype(f32)
    p["wprojT"] = np.ascontiguousarray((proj_w * ls2[:, None]).T).astype(BF_NP)
    p["projb_s"] = np.ascontiguousarray(
        (ls2 * np.asarray(inp["proj_b"], f32)).reshape(8, 128).T
    )
    p["gattn_s"] = np.ascontiguousarray(
        (ls1 * np.asarray(inp["ln_attn_g"], f32)).reshape(8, 128).T
    )
    p["battn_s"] = np.ascontiguousarray(
        (ls1 * np.asarray(inp["ln_attn_b"], f32)).reshape(8, 128).T
    )

    lsc = np.exp(np.minimum(np.asarray(inp["logit_scale"], f32).reshape(H),
                            np.log(100.0)))
    onesq = np.zeros((128, 8, 128), f32)
    for mt in range(8):
        for blk in range(2):
            h = 2 * mt + blk
            onesq[64 * blk:64 * (blk + 1), mt,
                  64 * blk:64 * (blk + 1)] = 1.0 / lsc[h] ** 2
    p["onesq"] = np.ascontiguousarray(onesq.reshape(128, 1024)).astype(BF_NP)
    onesk = np.zeros((128, 128), f32)
    onesk[:64, :64] = 1.0
    onesk[64:, 64:] = 1.0
    p["onesk"] = onesk.astype(BF_NP)
    hs = np.asarray(inp["head_scale"], f32)
    invhs = np.zeros((1, C), f32)
    for h in range(H):
        invhs[0, HD * h:HD * (h + 1)] = 1.0 / hs[h]
    p["invhs"] = invhs.astype(BF_NP)
    return p


def kernel(**inputs) -> np.ndarray:
    global _NC_CACHE, LAST_EXEC_NS, LAST_RESULTS
    if _NC_CACHE is None:
        _NC_CACHE = _build()
    nc = _NC_CACHE

    p = _host_prep(inputs)
    x = np.asarray(inputs["x"], np.float32)

    in_maps = []
    for c in range(N_CORES):
        b, r = c // RANKS, c % RANKS
        m = dict(p)
        xs = np.ascontiguousarray(x[b, T * r:T * (r + 1), :].T)   # [C, T]
        m["xTf"] = xs
        m["xTb"] = xs.astype(BF_NP)
        in_maps.append(m)

    kwargs = {}
    if TRACE:
        import os
        os.makedirs(TRACE_DIR, exist_ok=True)
        kwargs = dict(trace=True, tmpdir=TRACE_DIR)
    res = run_bass_kernel_spmd(nc, in_maps, list(range(N_CORES)), **kwargs)
    LAST_EXEC_NS = res.exec_time_ns
    LAST_RESULTS = res
    out = np.zeros((B, L, C), np.float32)
    for c in range(N_CORES):
        b, r = c // RANKS, c % RANKS
        out[b, T * r:T * (r + 1), :] = res.results[c]["outT"].T
    return out


# revision 15
# speedup vs baseline: 1.0622x; 1.0335x over previous
"""Trainium2 Bass kernel for nn_CustomResidualAttentionBlock (open_clip-style block).

Sharding: sequence-parallel over 8 cores. Core c owns 512 tokens
(b = c // 4, tokens [512*(c%4) : 512*(c%4+1)]). Each core computes q/k/v for its
own tokens, l2-normalizes k (and q) locally via a ones-block matmul trick, then
two AllGathers per 4-core batch group distribute (kT, v) for the full 2048-key
sequence. Attention, out-proj, residuals and the MLP are fully local.

The kernel is transpose-free: the host ships x already transposed (xT), all
layernorms are applied as rank-1 corrections after the matmuls
(y_ln = rstd*(raw - mu*colsum(W)) + b), with LN statistics computed in the
transposed layout via ones-vector matmuls and partition-replicated via K=1
matmuls. Activations flow as: xT -> qkT/kT/v -> attention (S^T, P^T, O^T) ->
yT -> uT -> h1T -> fcT -> gT -> outT; the host un-transposes the output.

Host-side folds (exact math, fp32):
  - ln1_g into wqkT/wvT columns; ln1_b @ W^T into the qkv biases
  - ln2_g into wfcT; ln2_b @ fc_w^T into fc bias
  - ls1 into ln_attn affine (g' = ls1*g, b' = ls1*b)
  - ls2 into proj weights/bias
  - logit_scale (clamped+exp'd) into the q-norm ones-block (1/lsc^2 entries)
  - head_scale into the rowsum-replication lhsT (1/hs entries)
  - column sums of the (bf16) qkv weights for the LN rank-1 correction

All big matmuls run in bf16 with fp32 PSUM accumulation; layernorm statistics,
softmax row sums and normalization factors stay in fp32.
"""
import numpy as np
import ml_dtypes

import concourse.bass as bass
import concourse.mybir as mybir
import concourse.tile as tile
from concourse import bacc
from concourse.bass_utils import run_bass_kernel_spmd

F32 = mybir.dt.float32
BF16 = mybir.dt.bfloat16
BF_NP = ml_dtypes.bfloat16
AF = mybir.ActivationFunctionType
ALU = mybir.AluOpType

B, L, C, H = 2, 2048, 1024, 16
HD = C // H          # 64
MLP = 4 * C          # 4096
N_CORES = 8
RANKS = 4            # cores per batch group
T = (B * L) // N_CORES  # 512 own tokens per core
TT = T // 128        # 4 token tiles
CT = C // 128        # 8 channel tiles
HP = H // 2          # 8 head pairs
KM = L // 128        # 16 key chunks
MT_FC = MLP // 128   # 32
LN_EPS = 1e-5
N_FC_PRE = 3

TRACE = False
TRACE_DIR = "/tmp/bass_trace"
LAST_EXEC_NS = None
LAST_RESULTS = None

_NC_CACHE = None


def _build():
    nc = bacc.Bacc(None, target_bir_lowering=False, debug=False, num_devices=N_CORES)

    # ---- I/O ----
    xtf_d = nc.dram_tensor("xTf", [C, T], F32, kind="ExternalInput")
    xtb_d = nc.dram_tensor("xTb", [C, T], BF16, kind="ExternalInput")
    out_d = nc.dram_tensor("outT", [C, T], F32, kind="ExternalOutput")
    wqk_d = nc.dram_tensor("wqkT", [16, 128, CT, 128], BF16, kind="ExternalInput")
    wv_d = nc.dram_tensor("wvT", [C, C], BF16, kind="ExternalInput")
    wo_d = nc.dram_tensor("woT", [C, C], BF16, kind="ExternalInput")
    wfc_d = nc.dram_tensor("wfcT", [MT_FC, 128, CT, 128], BF16, kind="ExternalInput")
    wpj_d = nc.dram_tensor("wprojT", [MLP, C], BF16, kind="ExternalInput")
    qkb_d = nc.dram_tensor("qkb", [128, 16], F32, kind="ExternalInput")
    cqk_d = nc.dram_tensor("csumqk", [1, 2 * C], F32, kind="ExternalInput")
    vb_d = nc.dram_tensor("vb", [1, C], F32, kind="ExternalInput")
    cv_d = nc.dram_tensor("csumv", [1, C], F32, kind="ExternalInput")
    outb_d = nc.dram_tensor("outb_s", [128, 8], F32, kind="ExternalInput")
    fcb_d = nc.dram_tensor("fcb", [128, 32], F32, kind="ExternalInput")
    pjb_d = nc.dram_tensor("projb_s", [128, 8], F32, kind="ExternalInput")
    ga_d = nc.dram_tensor("gattn_s", [128, 8], F32, kind="ExternalInput")
    ba_d = nc.dram_tensor("battn_s", [128, 8], F32, kind="ExternalInput")
    onesq_d = nc.dram_tensor("onesq", [128, 8 * 128], BF16, kind="ExternalInput")
    onesk_d = nc.dram_tensor("onesk", [128, 128], BF16, kind="ExternalInput")
    invhs_d = nc.dram_tensor("invhs", [1, C], BF16, kind="ExternalInput")

    with tile.TileContext(nc) as tc:
        with (
            tc.tile_pool(name="cn", bufs=1) as cn,
            tc.tile_pool(name="mid", bufs=1) as mid,
            tc.tile_pool(name="dram", bufs=1, space="DRAM") as dram,
        ):
            # ---- persistent activations ----
            xTf = [mid.tile([128, T], F32, name=f"xTf{c}") for c in range(CT)]
            xTb = [mid.tile([128, T], BF16, name=f"xTb{c}") for c in range(CT)]
            for ct in range(CT):
                nc.sync.dma_start(out=xTb[ct][:],
                                  in_=xtb_d[128 * ct:128 * (ct + 1), :])
            qTn = [mid.tile([128, T], BF16, name=f"qTn{p}") for p in range(HP)]
            OT = [mid.tile([128, T], BF16, name=f"OT{p}") for p in range(HP)]
            uT = [mid.tile([128, T], F32, name=f"uT{c}") for c in range(CT)]
            h1T = [mid.tile([128, T], BF16, name=f"h1T{c}") for c in range(CT)]
            wo_sb = mid.tile([128, CT, C], BF16)
            v_ag = [mid.tile([128, 4, H, HD + 1], BF16, name=f"vag{r}")
                    for r in range(RANKS)]
            wfc_pre = [mid.tile([128, CT, 128], BF16, name=f"wfcp{m}")
                       for m in range(N_FC_PRE)]

            # ---- small constants ----
            eps_sb = cn.tile([128, 1], F32)
            nc.vector.memset(eps_sb[:], LN_EPS)
            ones_c = cn.tile([128, 1], F32)
            nc.vector.memset(ones_c[:], 1.0)
            ones_cb = cn.tile([128, 1], BF16)
            nc.vector.memset(ones_cb[:], 1.0)
            ones_r = cn.tile([1, 128], F32)
            nc.vector.memset(ones_r[:], 1.0)
            qkb_sb = cn.tile([128, 16], F32)
            nc.sync.dma_start(out=qkb_sb[:], in_=qkb_d[:])
            cqk_sb = cn.tile([1, 2 * C], F32)
            nc.sync.dma_start(out=cqk_sb[:], in_=cqk_d[:])
            cv_row = cn.tile([1, C], F32)
            nc.sync.dma_start(out=cv_row[:], in_=cv_d[:])
            onesq_sb = cn.tile([128, 8, 128], BF16)
            nc.sync.dma_start(
                out=onesq_sb[:], in_=onesq_d[:].rearrange("p (m j) -> p m j", j=128)
            )
            onesk_sb = cn.tile([128, 128], BF16)
            nc.sync.dma_start(out=onesk_sb[:], in_=onesk_d[:])
            invhs_sb = cn.tile([1, C], BF16)
            nc.sync.dma_start(out=invhs_sb[:], in_=invhs_d[:])
            fcb_sb = cn.tile([128, 32], F32)
            nc.sync.dma_start(out=fcb_sb[:], in_=fcb_d[:])
            outb_sb = cn.tile([128, 8], F32)
            nc.sync.dma_start(out=outb_sb[:], in_=outb_d[:])
            pjb_sb = cn.tile([128, 8], F32)
            nc.sync.dma_start(out=pjb_sb[:], in_=pjb_d[:])
            ga_sb = cn.tile([128, 8], F32)
            nc.sync.dma_start(out=ga_sb[:], in_=ga_d[:])
            ba_sb = cn.tile([128, 8], F32)
            nc.sync.dma_start(out=ba_sb[:], in_=ba_d[:])
            vb_bc = cn.tile([128, C], F32)
            nc.sync.dma_start(out=vb_bc[:], in_=vb_d[:].to_broadcast([128, C]))

            # ---- AG buffers (k and v gathered separately) ----
            bounce_k = dram.tile([1024, T], BF16)
            ag_k = dram.tile([4096, T], BF16)
            bounce_v = dram.tile([1024, T], BF16)
            ag_v = dram.tile([4096, T], BF16)
            stat_d = dram.tile([2, T], F32)   # rstd / mu*rstd token-major bounce

            def ln_stats_T(tiles, rot, psp):
                """LN stats over the partition (channel) axis of 8 [128, T] tiles.

                Returns (a_rep, c_rep, rstd, murstd): rstd and mu*rstd replicated
                to 128 partitions (fp32 sbuf), and the [1, T] fp32 row versions.
                """
                pmean = psp.tile([1, T], F32, tag="stat", bufs=2, name="pmean")
                pvar = psp.tile([1, T], F32, tag="stat", bufs=2, name="pvar")
                ones_v = ones_cb if tiles[0].dtype == BF16 else ones_c
                for ct in range(CT):
                    nc.tensor.matmul(pmean[:], ones_v[:], tiles[ct][:],
                                     start=(ct == 0), stop=(ct == CT - 1))
                for ct in range(CT):
                    sq = rot.tile([128, T], BF16, tag="lnsq", name=f"lnsq{ct}")
                    nc.scalar.activation(out=sq[:], in_=tiles[ct][:], func=AF.Square)
                    nc.tensor.matmul(pvar[:], ones_cb[:], sq[:],
                                     start=(ct == 0), stop=(ct == CT - 1))
                mu = rot.tile([1, T], F32, tag="lnmu", bufs=1, name="mu")
                nc.vector.tensor_scalar(out=mu[:], in0=pmean[:], scalar1=1.0 / C,
                                        scalar2=None, op0=ALU.mult)
                m2 = rot.tile([1, T], F32, tag="lnm2", bufs=1, name="m2")
                nc.vector.tensor_mul(out=m2[:], in0=mu[:], in1=mu[:])
                var = rot.tile([1, T], F32, tag="lnvar", bufs=1, name="var")
                # var = E[x^2] - mu^2
                nc.vector.scalar_tensor_tensor(
                    out=var[:], in0=pvar[:], scalar=1.0 / C, in1=m2[:],
                    op0=ALU.mult, op1=ALU.subtract,
                )
                rstd = rot.tile([1, T], F32, tag="lnrstd", bufs=1, name="rstd")
                nc.scalar.activation(out=rstd[:], in_=var[:], func=AF.Sqrt,
                                     bias=eps_sb[0:1, :])
                nc.vector.reciprocal(out=rstd[:], in_=rstd[:])
                murstd = rot.tile([1, T], F32, tag="lnmr", bufs=1, name="murstd")
                nc.vector.tensor_mul(out=murstd[:], in0=mu[:], in1=rstd[:])
                negmu = rot.tile([1, T], F32, tag="lnnm", bufs=1, name="negmu")
                nc.vector.tensor_scalar(out=negmu[:], in0=mu[:], scalar1=-1.0,
                                        scalar2=None, op0=ALU.mult)
                # replicate to 128 partitions via K=1 matmuls
                prep = psp.tile([128, T], F32, tag="repl", bufs=1, name="prep")
                a_rep = rot.tile([128, T], F32, tag="arep", bufs=1, name="a_rep")
                nc.tensor.matmul(prep[:], ones_r[:], rstd[:], start=True, stop=True)
                nc.vector.tensor_copy(out=a_rep[:], in_=prep[:])
                prep2 = psp.tile([128, T], F32, tag="repl", bufs=1, name="prep2")
                c_rep = rot.tile([128, T], F32, tag="crep", bufs=1, name="c_rep")
                nc.tensor.matmul(prep2[:], ones_r[:], murstd[:], start=True,
                                 stop=True)
                nc.vector.tensor_copy(out=c_rep[:], in_=prep2[:])
                return a_rep, c_rep, rstd, murstd, negmu

            with (
                tc.tile_pool(name="pa", bufs=1) as pa,
                tc.tile_pool(name="rot1", bufs=2) as rot1,
                tc.tile_pool(name="ps1", bufs=1, space="PSUM") as ps1,
            ):
                # ---- phase 1: LN1 statistics (transposed layout) ----
                a1_rep, c1_rep, rstd1, murstd1, negmu1 = ln_stats_T(xTb, rot1, ps1)
                # bounce (rstd, murstd) through DRAM to get them token-major
                nc.scalar.dma_start(out=stat_d[0:1, :], in_=rstd1[:])
                stat_cols = cn.tile([128, TT], F32)
                nc.scalar.dma_start(
                    out=stat_cols[:],
                    in_=stat_d[0:1, :].rearrange("j (t p) -> p (j t)", p=128),
                )

                def qk_tile(mt):
                    """Project + LN-correct + l2-normalize one qk row-tile."""
                    wqk_t = rot1.tile([128, CT, 128], BF16, tag="wqk", bufs=3,
                                      name=f"wqk{mt}")
                    nc.sync.dma_start(out=wqk_t[:], in_=wqk_d[mt])
                    pqk = ps1.tile([128, T], F32, tag="mm", bufs=3, name=f"pqk{mt}")
                    for kc in range(CT):
                        nc.tensor.matmul(
                            pqk[:], wqk_t[:, kc, :], xTb[kc][:],
                            start=(kc == 0), stop=False,
                        )
                    # fold the -mu*colsum(W) LN correction into the accumulation
                    nc.tensor.matmul(
                        pqk[:], cqk_sb[0:1, 128 * mt:128 * (mt + 1)], negmu1[:],
                        start=False, stop=True,
                    )
                    # qk = rstd*(raw - mu*csum) + bias
                    t1 = rot1.tile([128, T], F32, tag="t1", bufs=2, name=f"t1_{mt}")
                    nc.vector.tensor_mul(out=t1[:], in0=pqk[:], in1=a1_rep[:])
                    qk_f = rot1.tile([128, T], F32, tag="qkf", name=f"qkf{mt}")
                    nc.vector.tensor_scalar(out=qk_f[:], in0=t1[:],
                                            scalar1=qkb_sb[:, mt:mt + 1],
                                            scalar2=None, op0=ALU.add)
                    sq = rot1.tile([128, T], BF16, tag="sq", bufs=3,
                                   name=f"sq{mt}")
                    nc.scalar.activation(out=sq[:], in_=qk_f[:], func=AF.Square)
                    pn = ps1.tile([128, T], F32, tag="nrm", bufs=2, name=f"pn{mt}")
                    ones = onesq_sb[:, mt, :] if mt < 8 else onesk_sb[:]
                    nc.tensor.matmul(pn[:], ones, sq[:], start=True, stop=True)
                    sq2 = rot1.tile([128, T], F32, tag="sqn", bufs=2, name=f"sqn{mt}")
                    nc.scalar.activation(out=sq2[:], in_=pn[:], func=AF.Sqrt)
                    rrec = rot1.tile([128, T], F32, tag="rrec", name=f"rrec{mt}")
                    nc.vector.reciprocal(out=rrec[:], in_=sq2[:])
                    return qk_f, rrec

                # ---- phase 3: v (token-major) -> bounce -> AG-v ----
                wv_sb = pa.tile([128, CT, C], BF16)
                nc.sync.dma_start(
                    out=wv_sb[:], in_=wv_d[:].rearrange("(k p) m -> p k m", p=128)
                )
                for t in range(TT):
                    v_sb = rot1.tile([128, C], BF16, tag="vsb", bufs=1, name=f"vsb{t}")
                    rstd_c = stat_cols[:, t:t + 1]
                    for n2 in range(2):
                        pv = ps1.tile([128, 512], F32, tag="mm", bufs=3,
                                      name=f"pv{t}{n2}")
                        for kc in range(CT):
                            nc.tensor.matmul(
                                pv[:],
                                xTb[kc][:, 128 * t:128 * (t + 1)],
                                wv_sb[:, kc, 512 * n2:512 * (n2 + 1)],
                                start=(kc == 0), stop=False,
                            )
                        nc.tensor.matmul(
                            pv[:],
                            negmu1[0:1, 128 * t:128 * (t + 1)],
                            cv_row[0:1, 512 * n2:512 * (n2 + 1)],
                            start=False, stop=True,
                        )
                        # v = rstd*(raw - mu*csumv) + vb
                        nc.vector.scalar_tensor_tensor(
                            out=v_sb[:, 512 * n2:512 * (n2 + 1)], in0=pv[:],
                            scalar=rstd_c,
                            in1=vb_bc[:, 512 * n2:512 * (n2 + 1)],
                            op0=ALU.mult, op1=ALU.add,
                        )
                    nc.gpsimd.dma_start(
                        out=bounce_v[256 * t:256 * (t + 1), :].rearrange(
                            "(p a) f -> p (a f)", p=128
                        ),
                        in_=v_sb[:],
                    )
                nc.gpsimd.collective_compute(
                    "AllGather", ALU.bypass,
                    replica_groups=[[0, 1, 2, 3], [4, 5, 6, 7]],
                    ins=[bounce_v.opt()], outs=[ag_v.opt()],
                )

                # ---- phase 2k: k rows -> bounce -> AG-k ----
                for mt in range(8, 16):
                    qk_f, rrec = qk_tile(mt)
                    i = mt - 8
                    ktn = rot1.tile([128, T], BF16, tag="ktn", bufs=2, name=f"ktn{i}")
                    nc.vector.tensor_mul(out=ktn[:], in0=qk_f[:], in1=rrec[:])
                    nc.gpsimd.dma_start(
                        out=bounce_k[128 * i:128 * (i + 1), :], in_=ktn[:]
                    )
                nc.gpsimd.collective_compute(
                    "AllGather", ALU.bypass,
                    replica_groups=[[0, 1, 2, 3], [4, 5, 6, 7]],
                    ins=[bounce_k.opt()], outs=[ag_k.opt()],
                )

                # prefetches (all data-ready at issue): out-proj weights,
                # residual xTf, first fc weights
                nc.sync.dma_start(
                    out=wo_sb[:], in_=wo_d[:].rearrange("(k p) m -> p k m", p=128)
                )
                for ct in range(CT):
                    nc.sync.dma_start(out=xTf[ct][:],
                                      in_=xtf_d[128 * ct:128 * (ct + 1), :])
                for mt in range(N_FC_PRE):
                    nc.sync.dma_start(out=wfc_pre[mt][:], in_=wfc_d[mt])

                # ---- phase 2q: q rows ----
                for mt in range(8):
                    qk_f, rrec = qk_tile(mt)
                    nc.vector.tensor_mul(out=qTn[mt][:], in0=qk_f[:], in1=rrec[:])

            # ---- phase 4: attention ----
            with (
                tc.tile_pool(name="pb", bufs=1) as pb,
                tc.tile_pool(name="rot2", bufs=2) as rot2,
                tc.tile_pool(name="ps2", bufs=1, space="PSUM") as ps2,
            ):
                for r in range(RANKS):
                    nc.vector.memset(v_ag[r][:], 1.0)
                    for sub in range(4):
                        vsrc = ag_v[
                            1024 * r + 256 * sub:1024 * r + 256 * (sub + 1), :
                        ].rearrange("(p a) f -> p (a f)", p=128).rearrange(
                            "p (h d) -> p h d", d=HD
                        )
                        nc.scalar.dma_start(out=v_ag[r][:, sub, :, 0:HD], in_=vsrc)
                ag_k_r = ag_k[:].rearrange("(r m p) f -> p r m f", r=RANKS, p=128)
                for hp in range(HP):
                    kpair = pb.tile([128, RANKS, T], BF16, tag="kpair", bufs=3,
                                    name=f"kpair{hp}")
                    nc.scalar.dma_start(out=kpair[:], in_=ag_k_r[:, :, hp, :])
                    po = [
                        ps2.tile([HD + 1, T], F32, tag=f"o{h2}", bufs=2,
                                 name=f"po{h2}")
                        for h2 in range(2)
                    ]
                    for km in range(KM):
                        r, sub = km // 4, km % 4
                        psS = ps2.tile([128, 2, T], F32, tag="s", bufs=2,
                                       name=f"psS{km}")
                        for h2 in range(2):
                            nc.tensor.matmul(
                                psS[:, h2, :],
                                kpair[64 * h2:64 * (h2 + 1), r,
                                      128 * sub:128 * (sub + 1)],
                                qTn[hp][64 * h2:64 * (h2 + 1), :],
                                start=True, stop=True,
                            )
                        pT = rot2.tile([128, 2, T], BF16, tag="pT", bufs=4,
                                       name=f"pT{km}")
                        nc.scalar.activation(out=pT[:], in_=psS[:], func=AF.Exp)
                        for h2 in range(2):
                            h = 2 * hp + h2
                            nc.tensor.matmul(
                                po[h2][:],
                                v_ag[r][:, sub, h, :],
                                pT[:, h2, :],
                                start=(km == 0), stop=(km == KM - 1),
                            )
                    for h2 in range(2):
                        h = 2 * hp + h2
                        rs = rot2.tile([1, T], BF16, tag="rs")
                        nc.vector.tensor_copy(out=rs[:], in_=po[h2][HD:HD + 1, :])
                        pr = ps2.tile([128, 2, T], F32, tag="s", bufs=2,
                                      name=f"pr{h2}")
                        nc.tensor.matmul(
                            pr[0:HD, 0, :],
                            invhs_sb[0:1, HD * h:HD * (h + 1)], rs[:],
                            start=True, stop=True,
                        )
                        rrec = rot2.tile([HD, T], F32, tag="orec")
                        nc.vector.reciprocal(out=rrec[:], in_=pr[0:HD, 0, :])
                        nc.vector.tensor_mul(
                            out=OT[hp][64 * h2:64 * (h2 + 1), :],
                            in0=po[h2][0:HD, :],
                            in1=rrec[:],
                        )

            # ---- phase 5: out projection (transposed out) + ln_attn + ln2 ----
            with (
                tc.tile_pool(name="rot3", bufs=2) as rot3,
                tc.tile_pool(name="ps3", bufs=1, space="PSUM") as ps3,
            ):
                yT = [rot3.tile([128, T], F32, tag="yT", bufs=8, name=f"yT{m}")
                      for m in range(CT)]
                for m in range(CT):
                    py = ps3.tile([128, T], F32, tag="y", bufs=3, name=f"py{m}")
                    for kc in range(CT):
                        nc.tensor.matmul(
                            py[:],
                            wo_sb[:, kc, 128 * m:128 * (m + 1)],
                            OT[kc][:],
                            start=(kc == 0), stop=(kc == CT - 1),
                        )
                    nc.vector.tensor_scalar(out=yT[m][:], in0=py[:],
                                            scalar1=outb_sb[:, m:m + 1],
                                            scalar2=None, op0=ALU.add)
                a2_rep, c2_rep, _, _, _ = ln_stats_T(yT, rot3, ps3)
                # uT = xT + (yT - mu)*rstd*g' + b'   (g', b' have ls1 folded)
                for ct in range(CT):
                    t1 = rot3.tile([128, T], F32, tag="ut1", name=f"ut1_{ct}")
                    nc.vector.tensor_mul(out=t1[:], in0=yT[ct][:], in1=a2_rep[:])
                    t2 = rot3.tile([128, T], F32, tag="ut2", name=f"ut2_{ct}")
                    nc.vector.tensor_sub(out=t2[:], in0=t1[:], in1=c2_rep[:])
                    t3 = rot3.tile([128, T], F32, tag="ut3", name=f"ut3_{ct}")
                    nc.vector.tensor_scalar(out=t3[:], in0=t2[:],
                                            scalar1=ga_sb[:, ct:ct + 1],
                                            scalar2=ba_sb[:, ct:ct + 1],
                                            op0=ALU.mult, op1=ALU.add)
                    nc.vector.tensor_add(out=uT[ct][:], in0=t3[:], in1=xTf[ct][:])
                a3_rep, c3_rep, _, _, _ = ln_stats_T(uT, rot3, ps3)
                for ct in range(CT):
                    # h1T = (uT)*rstd - mu*rstd  (ln2 affine folded into fc)
                    t4 = rot3.tile([128, T], F32, tag="ut4", name=f"ut4_{ct}")
                    nc.vector.tensor_mul(out=t4[:], in0=uT[ct][:], in1=a3_rep[:])
                    nc.vector.tensor_sub(out=h1T[ct][:], in0=t4[:], in1=c3_rep[:])

            # ---- phase 7: fc + gelu;  phase 8: proj (transposed) + residual ----
            with (
                tc.tile_pool(name="pd", bufs=1) as pd,
                tc.tile_pool(name="rot4", bufs=2) as rot4,
                tc.tile_pool(name="ps4", bufs=1, space="PSUM") as ps4,
            ):
                gT = [pd.tile([128, T], BF16, name=f"gT{m}") for m in range(MT_FC)]
                for mt in range(MT_FC):
                    if mt < N_FC_PRE:
                        wfc_t = wfc_pre[mt]
                    else:
                        wfc_t = rot4.tile([128, CT, 128], BF16, tag="wfc", bufs=6,
                                          name=f"wfc{mt}")
                        nc.sync.dma_start(out=wfc_t[:], in_=wfc_d[mt])
                    pfc = ps4.tile([128, T], F32, tag="fc", bufs=3, name=f"pfc{mt}")
                    for kc in range(CT):
                        nc.tensor.matmul(
                            pfc[:], wfc_t[:, kc, :], h1T[kc][:],
                            start=(kc == 0), stop=(kc == CT - 1),
                        )
                    nc.scalar.activation(out=gT[mt][:], in_=pfc[:], func=AF.Gelu,
                                         bias=fcb_sb[:, mt:mt + 1])

                for half in range(2):
                    ppj = [
                        ps4.tile([128, T], F32, tag=f"pj{i}", bufs=1,
                                 name=f"ppj{i}")
                        for i in range(4)
                    ]
                    for mt in range(MT_FC):
                        wpj_t = rot4.tile([128, 4, 128], BF16, tag="wpj", bufs=8,
                                          name=f"wpj{half}_{mt}")
                        nc.sync.dma_start(
                            out=wpj_t[:],
                            in_=wpj_d[128 * mt:128 * (mt + 1),
                                      512 * half:512 * (half + 1)].rearrange(
                                "p (i m) -> p i m", m=128
                            ),
                        )
                        for i in range(4):
                            nc.tensor.matmul(
                                ppj[i][:],
                                wpj_t[:, i, :],
                                gT[mt][:],
                                start=(mt == 0), stop=(mt == MT_FC - 1),
                            )
                    for i in range(4):
                        ct = 4 * half + i
                        o1 = rot4.tile([128, T], F32, tag="ofin",
                                       name=f"of{half}_{i}")
                        nc.vector.tensor_scalar(out=o1[:], in0=ppj[i][:],
                                                scalar1=pjb_sb[:, ct:ct + 1],
                                                scalar2=None, op0=ALU.add)
                        nc.vector.tensor_add(out=o1[:], in0=o1[:], in1=uT[ct][:])
                        nc.gpsimd.dma_start(
                            out=out_d[128 * ct:128 * (ct + 1), :], in_=o1[:]
                        )

    nc.compile()
    return nc


def _host_prep(inp):
    f32 = np.float32
    ln1_g = np.asarray(inp["ln1_g"], f32)
    ln1_b = np.asarray(inp["ln1_b"], f32)
    ln2_g = np.asarray(inp["ln2_g"], f32)
    ln2_b = np.asarray(inp["ln2_b"], f32)
    in_w = np.asarray(inp["in_proj_w"], f32)
    in_b = np.asarray(inp["in_proj_b"], f32)
    fc_w = np.asarray(inp["fc_w"], f32)
    proj_w = np.asarray(inp["proj_w"], f32)
    ls1 = np.asarray(inp["ls1"], f32)
    ls2 = np.asarray(inp["ls2"], f32)

    w_qk = in_w[:2 * C]
    w_v = in_w[2 * C:]
    p = {}
    wqkT = np.ascontiguousarray((w_qk * ln1_g[None, :]).T).astype(BF_NP)
    wvT = np.ascontiguousarray((w_v * ln1_g[None, :]).T).astype(BF_NP)
    p["wqkT"] = np.ascontiguousarray(
        wqkT.reshape(CT, 128, 16, 128).transpose(2, 1, 0, 3)
    )
    p["wvT"] = wvT
    # column sums of the bf16 weights, as the device matmul sees them
    csum_qk = wqkT.astype(f32).sum(0)
    csum_v = wvT.astype(f32).sum(0)
    p["csumqk"] = csum_qk.reshape(1, 2 * C).astype(f32)
    p["csumv"] = csum_v.reshape(1, C).astype(f32)
    qkb = in_b[:2 * C] + ln1_b @ w_qk.T
    p["qkb"] = np.ascontiguousarray(qkb.reshape(16, 128).T).astype(f32)
    p["vb"] = (in_b[2 * C:] + ln1_b @ w_v.T).reshape(1, C).astype(f32)
    p["woT"] = np.ascontiguousarray(np.asarray(inp["out_w"], f32).T).astype(BF_NP)
    p["outb_s"] = np.ascontiguousarray(
        np.asarray(inp["out_b"], f32).reshape(8, 128).T
    )
    wfcT = np.ascontiguousarray((fc_w * ln2_g[None, :]).T).astype(BF_NP)
    p["wfcT"] = np.ascontiguousarray(
        wfcT.reshape(CT, 128, MT_FC, 128).transpose(2, 1, 0, 3)
    )
    fcb = np.asarray(inp["fc_b"], f32) + ln2_b @ fc_w.T
    p["fcb"] = np.ascontiguousarray(fcb.reshape(32, 128).T).astype(f32)
    p["wprojT"] = np.ascontiguousarray((proj_w * ls2[:, None]).T).astype(BF_NP)
    p["projb_s"] = np.ascontiguousarray(
        (ls2 * np.asarray(inp["proj_b"], f32)).reshape(8, 128).T
    )
    p["gattn_s"] = np.ascontiguousarray(
        (ls1 * np.asarray(inp["ln_attn_g"], f32)).reshape(8, 128).T
    )
    p["battn_s"] = np.ascontiguousarray(
        (ls1 * np.asarray(inp["ln_attn_b"], f32)).reshape(8, 128).T
    )

    lsc = np.exp(np.minimum(np.asarray(inp["logit_scale"], f32).reshape(H),
                            np.log(100.0)))
    onesq = np.zeros((128, 8, 128), f32)
    for mt in range(8):
        for blk in range(2):
            h = 2 * mt + blk
            onesq[64 * blk:64 * (blk + 1), mt,
                  64 * blk:64 * (blk + 1)] = 1.0 / lsc[h] ** 2
    p["onesq"] = np.ascontiguousarray(onesq.reshape(128, 1024)).astype(BF_NP)
    onesk = np.zeros((128, 128), f32)
    onesk[:64, :64] = 1.0
    onesk[64:, 64:] = 1.0
    p["onesk"] = onesk.astype(BF_NP)
    hs = np.asarray(inp["head_scale"], f32)
    invhs = np.zeros((1, C), f32)
    for h in range(H):
        invhs[0, HD * h:HD * (h + 1)] = 1.0 / hs[h]
    p["invhs"] = invhs.astype(BF_NP)
    return p


def kernel(**inputs) -> np.ndarray:
    global _NC_CACHE, LAST_EXEC_NS, LAST_RESULTS
    if _NC_CACHE is None:
        _NC_CACHE = _build()
    nc = _NC_CACHE

    p = _host_prep(inputs)
    x = np.asarray(inputs["x"], np.float32)

    in_maps = []
    for c in range(N_CORES):
        b, r = c // RANKS, c % RANKS
        m = dict(p)
        xs = np.ascontiguousarray(x[b, T * r:T * (r + 1), :].T)   # [C, T]
        m["xTf"] = xs
        m["xTb"] = xs.astype(BF_NP)
        in_maps.append(m)

    kwargs = {}
    if TRACE:
        import os
        os.makedirs(TRACE_DIR, exist_ok=True)
        kwargs = dict(trace=True, tmpdir=TRACE_DIR)
    res = run_bass_kernel_spmd(nc, in_maps, list(range(N_CORES)), **kwargs)
    LAST_EXEC_NS = res.exec_time_ns
    LAST_RESULTS = res
    out = np.zeros((B, L, C), np.float32)
    for c in range(N_CORES):
        b, r = c // RANKS, c % RANKS
        out[b, T * r:T * (r + 1), :] = res.results[c]["outT"].T
    return out


# revision 17
# speedup vs baseline: 1.1712x; 1.1027x over previous
"""Trainium2 Bass kernel for nn_CustomResidualAttentionBlock (open_clip-style block).

Sharding: sequence-parallel over 8 cores. Core c owns 512 tokens
(b = c // 4, tokens [512*(c%4) : 512*(c%4+1)]). Each core computes q/k/v for its
own tokens, l2-normalizes k (and q) locally via a ones-block matmul trick, then
two AllGathers per 4-core batch group distribute (kT, v) for the full 2048-key
sequence. Attention, out-proj, residuals and the MLP are fully local.

The kernel is transpose-free: the host ships x already transposed (xT), all
layernorms are applied as rank-1 corrections after the matmuls
(y_ln = rstd*(raw - mu*colsum(W)) + b), with LN statistics computed in the
transposed layout via ones-vector matmuls and partition-replicated via K=1
matmuls. Activations flow as: xT -> qkT/kT/v -> attention (S^T, P^T, O^T) ->
yT -> uT -> h1T -> fcT -> gT -> outT; the host un-transposes the output.

Host-side folds (exact math, fp32):
  - ln1_g into wqkT/wvT columns; ln1_b @ W^T into the qkv biases
  - ln2_g into wfcT; ln2_b @ fc_w^T into fc bias
  - ls1 into ln_attn affine (g' = ls1*g, b' = ls1*b)
  - ls2 into proj weights/bias
  - logit_scale (clamped+exp'd) into the q-norm ones-block (1/lsc^2 entries)
  - head_scale into the rowsum-replication lhsT (1/hs entries)
  - column sums of the (bf16) qkv weights for the LN rank-1 correction

All big matmuls run in bf16 with fp32 PSUM accumulation; layernorm statistics,
softmax row sums and normalization factors stay in fp32.
"""
import numpy as np
import ml_dtypes

import concourse.bass as bass
import concourse.mybir as mybir
import concourse.tile as tile
from concourse import bacc
from concourse.bass_utils import run_bass_kernel_spmd

F32 = mybir.dt.float32
BF16 = mybir.dt.bfloat16
BF_NP = ml_dtypes.bfloat16
AF = mybir.ActivationFunctionType
ALU = mybir.AluOpType

B, L, C, H = 2, 2048, 1024, 16
HD = C // H          # 64
MLP = 4 * C          # 4096
N_CORES = 8
RANKS = 4            # cores per batch group
T = (B * L) // N_CORES  # 512 own tokens per core
TT = T // 128        # 4 token tiles
CT = C // 128        # 8 channel tiles
HP = H // 2          # 8 head pairs
KM = L // 128        # 16 key chunks
MT_FC = MLP // 128   # 32
LN_EPS = 1e-5
N_FC_PRE = 3

TRACE = False
TRACE_DIR = "/tmp/bass_trace"
LAST_EXEC_NS = None
LAST_RESULTS = None

_NC_CACHE = None


def _build():
    nc = bacc.Bacc(None, target_bir_lowering=False, debug=False, num_devices=N_CORES)

    # ---- I/O ----
    xtf_d = nc.dram_tensor("xTf", [C, T], F32, kind="ExternalInput")
    xtb_d = nc.dram_tensor("xTb", [C, T], BF16, kind="ExternalInput")
    out_d = nc.dram_tensor("outT", [C, T], F32, kind="ExternalOutput")
    wqk_d = nc.dram_tensor("wqkT", [16, 128, CT, 128], BF16, kind="ExternalInput")
    wv_d = nc.dram_tensor("wvT", [C, C], BF16, kind="ExternalInput")
    wo_d = nc.dram_tensor("woT", [C, C], BF16, kind="ExternalInput")
    wfc_d = nc.dram_tensor("wfcT", [MT_FC, 128, CT, 128], BF16, kind="ExternalInput")
    wpj_d = nc.dram_tensor("wprojT", [MLP, C], BF16, kind="ExternalInput")
    qkb_d = nc.dram_tensor("qkb", [128, 16], F32, kind="ExternalInput")
    cqk_d = nc.dram_tensor("csumqk", [1, 2 * C], F32, kind="ExternalInput")
    vb_d = nc.dram_tensor("vb", [1, C], F32, kind="ExternalInput")
    cv_d = nc.dram_tensor("csumv", [1, C], F32, kind="ExternalInput")
    outb_d = nc.dram_tensor("outb_s", [128, 8], F32, kind="ExternalInput")
    fcb_d = nc.dram_tensor("fcb", [128, 32], F32, kind="ExternalInput")
    pjb_d = nc.dram_tensor("projb_s", [128, 8], F32, kind="ExternalInput")
    ga_d = nc.dram_tensor("gattn_s", [128, 8], F32, kind="ExternalInput")
    ba_d = nc.dram_tensor("battn_s", [128, 8], F32, kind="ExternalInput")
    onesq_d = nc.dram_tensor("onesq", [128, 8 * 128], BF16, kind="ExternalInput")
    onesk_d = nc.dram_tensor("onesk", [128, 128], BF16, kind="ExternalInput")
    invhs_d = nc.dram_tensor("invhs", [1, C], BF16, kind="ExternalInput")

    with tile.TileContext(nc) as tc:
        with (
            tc.tile_pool(name="cn", bufs=1) as cn,
            tc.tile_pool(name="mid", bufs=1) as mid,
            tc.tile_pool(name="dram", bufs=1, space="DRAM") as dram,
        ):
            # ---- persistent activations ----
            xTf = [mid.tile([128, T], F32, name=f"xTf{c}") for c in range(CT)]
            xTb = [mid.tile([128, T], BF16, name=f"xTb{c}") for c in range(CT)]
            for ct in range(CT):
                nc.sync.dma_start(out=xTb[ct][:],
                                  in_=xtb_d[128 * ct:128 * (ct + 1), :])
            qTn = [mid.tile([128, T], BF16, name=f"qTn{p}") for p in range(HP)]
            OT = [mid.tile([128, T], BF16, name=f"OT{p}") for p in range(HP)]
            uT = [mid.tile([128, T], F32, name=f"uT{c}") for c in range(CT)]
            h1T = [mid.tile([128, T], BF16, name=f"h1T{c}") for c in range(CT)]
            wo_sb = mid.tile([128, CT, C], BF16)
            v_ag = [mid.tile([128, 4, H, HD + 1], BF16, name=f"vag{r}")
                    for r in range(RANKS)]
            wfc_pre = [mid.tile([128, CT, 128], BF16, name=f"wfcp{m}")
                       for m in range(N_FC_PRE)]

            # ---- small constants ----
            eps_sb = cn.tile([128, 1], F32)
            nc.vector.memset(eps_sb[:], LN_EPS)
            ones_c = cn.tile([128, 1], F32)
            nc.vector.memset(ones_c[:], 1.0)
            ones_cb = cn.tile([128, 1], BF16)
            nc.vector.memset(ones_cb[:], 1.0)
            ones_r = cn.tile([1, 128], F32)
            nc.vector.memset(ones_r[:], 1.0)
            qkb_sb = cn.tile([128, 16], F32)
            nc.sync.dma_start(out=qkb_sb[:], in_=qkb_d[:])
            cqk_sb = cn.tile([1, 2 * C], F32)
            nc.sync.dma_start(out=cqk_sb[:], in_=cqk_d[:])
            cv_row = cn.tile([1, C], F32)
            nc.sync.dma_start(out=cv_row[:], in_=cv_d[:])
            onesq_sb = cn.tile([128, 8, 128], BF16)
            nc.sync.dma_start(
                out=onesq_sb[:], in_=onesq_d[:].rearrange("p (m j) -> p m j", j=128)
            )
            onesk_sb = cn.tile([128, 128], BF16)
            nc.sync.dma_start(out=onesk_sb[:], in_=onesk_d[:])
            invhs_sb = cn.tile([1, C], BF16)
            nc.sync.dma_start(out=invhs_sb[:], in_=invhs_d[:])
            fcb_sb = cn.tile([128, 32], F32)
            nc.sync.dma_start(out=fcb_sb[:], in_=fcb_d[:])
            outb_sb = cn.tile([128, 8], F32)
            nc.sync.dma_start(out=outb_sb[:], in_=outb_d[:])
            pjb_sb = cn.tile([128, 8], F32)
            nc.sync.dma_start(out=pjb_sb[:], in_=pjb_d[:])
            ga_sb = cn.tile([128, 8], F32)
            nc.sync.dma_start(out=ga_sb[:], in_=ga_d[:])
            ba_sb = cn.tile([128, 8], F32)
            nc.sync.dma_start(out=ba_sb[:], in_=ba_d[:])
            vb_bc = cn.tile([128, C], F32)
            nc.sync.dma_start(out=vb_bc[:], in_=vb_d[:].to_broadcast([128, C]))

            # ---- AG buffers (k and v gathered separately) ----
            bounce_k = dram.tile([1024, T], BF16)
            ag_k = dram.tile([4096, T], BF16)
            bounce_v = dram.tile([1024, T], BF16)
            ag_v = dram.tile([4096, T], BF16)
            stat_d = dram.tile([2, T], F32)   # rstd / mu*rstd token-major bounce

            def ln_stats_T(tiles, rot, psp):
                """LN stats over the partition (channel) axis of 8 [128, T] tiles.

                Returns (a_rep, c_rep, rstd, murstd): rstd and mu*rstd replicated
                to 128 partitions (fp32 sbuf), and the [1, T] fp32 row versions.
                """
                pmean = psp.tile([1, T], F32, tag="stat", bufs=2, name="pmean")
                pvar = psp.tile([1, T], F32, tag="stat", bufs=2, name="pvar")
                ones_v = ones_cb if tiles[0].dtype == BF16 else ones_c
                for ct in range(CT):
                    nc.tensor.matmul(pmean[:], ones_v[:], tiles[ct][:],
                                     start=(ct == 0), stop=(ct == CT - 1))
                for ct in range(CT):
                    sq = rot.tile([128, T], BF16, tag="lnsq", name=f"lnsq{ct}")
                    nc.scalar.activation(out=sq[:], in_=tiles[ct][:], func=AF.Square)
                    nc.tensor.matmul(pvar[:], ones_cb[:], sq[:],
                                     start=(ct == 0), stop=(ct == CT - 1))
                mu = rot.tile([1, T], F32, tag="lnmu", bufs=1, name="mu")
                nc.vector.tensor_scalar(out=mu[:], in0=pmean[:], scalar1=1.0 / C,
                                        scalar2=None, op0=ALU.mult)
                m2 = rot.tile([1, T], F32, tag="lnm2", bufs=1, name="m2")
                nc.vector.tensor_mul(out=m2[:], in0=mu[:], in1=mu[:])
                var = rot.tile([1, T], F32, tag="lnvar", bufs=1, name="var")
                # var = E[x^2] - mu^2
                nc.vector.scalar_tensor_tensor(
                    out=var[:], in0=pvar[:], scalar=1.0 / C, in1=m2[:],
                    op0=ALU.mult, op1=ALU.subtract,
                )
                rstd = rot.tile([1, T], F32, tag="lnrstd", bufs=1, name="rstd")
                nc.scalar.activation(out=rstd[:], in_=var[:],
                                     func=AF.Abs_reciprocal_sqrt,
                                     bias=eps_sb[0:1, :])
                murstd = rot.tile([1, T], F32, tag="lnmr", bufs=1, name="murstd")
                nc.vector.tensor_mul(out=murstd[:], in0=mu[:], in1=rstd[:])
                negmu = rot.tile([1, T], F32, tag="lnnm", bufs=1, name="negmu")
                nc.vector.tensor_scalar(out=negmu[:], in0=mu[:], scalar1=-1.0,
                                        scalar2=None, op0=ALU.mult)
                # replicate to 128 partitions via K=1 matmuls
                prep = psp.tile([128, T], F32, tag="repl", bufs=1, name="prep")
                a_rep = rot.tile([128, T], F32, tag="arep", bufs=1, name="a_rep")
                nc.tensor.matmul(prep[:], ones_r[:], rstd[:], start=True, stop=True)
                nc.vector.tensor_copy(out=a_rep[:], in_=prep[:])
                prep2 = psp.tile([128, T], F32, tag="repl", bufs=1, name="prep2")
                c_rep = rot.tile([128, T], F32, tag="crep", bufs=1, name="c_rep")
                nc.tensor.matmul(prep2[:], ones_r[:], murstd[:], start=True,
                                 stop=True)
                nc.vector.tensor_copy(out=c_rep[:], in_=prep2[:])
                return a_rep, c_rep, rstd, murstd, negmu

            with (
                tc.tile_pool(name="pa", bufs=1) as pa,
                tc.tile_pool(name="rot1", bufs=2) as rot1,
                tc.tile_pool(name="ps1", bufs=1, space="PSUM") as ps1,
            ):
                # ---- phase 1: LN1 statistics (transposed layout) ----
                a1_rep, c1_rep, rstd1, murstd1, negmu1 = ln_stats_T(xTb, rot1, ps1)
                # bounce (rstd, murstd) through DRAM to get them token-major
                nc.scalar.dma_start(out=stat_d[0:1, :], in_=rstd1[:])
                stat_cols = cn.tile([128, TT], F32)
                nc.scalar.dma_start(
                    out=stat_cols[:],
                    in_=stat_d[0:1, :].rearrange("j (t p) -> p (j t)", p=128),
                )

                def qk_tile(mt):
                    """Project + LN-correct + l2-normalize one qk row-tile."""
                    wqk_t = rot1.tile([128, CT, 128], BF16, tag="wqk", bufs=3,
                                      name=f"wqk{mt}")
                    nc.sync.dma_start(out=wqk_t[:], in_=wqk_d[mt])
                    pqk = ps1.tile([128, T], F32, tag="mm", bufs=3, name=f"pqk{mt}")
                    for kc in range(CT):
                        nc.tensor.matmul(
                            pqk[:], wqk_t[:, kc, :], xTb[kc][:],
                            start=(kc == 0), stop=False,
                        )
                    # fold the -mu*colsum(W) LN correction into the accumulation
                    nc.tensor.matmul(
                        pqk[:], cqk_sb[0:1, 128 * mt:128 * (mt + 1)], negmu1[:],
                        start=False, stop=True,
                    )
                    # qk = rstd*(raw - mu*csum) + bias
                    t1 = rot1.tile([128, T], F32, tag="t1", bufs=2, name=f"t1_{mt}")
                    nc.vector.tensor_mul(out=t1[:], in0=pqk[:], in1=a1_rep[:])
                    qk_f = rot1.tile([128, T], F32, tag="qkf", name=f"qkf{mt}")
                    nc.vector.tensor_scalar(out=qk_f[:], in0=t1[:],
                                            scalar1=qkb_sb[:, mt:mt + 1],
                                            scalar2=None, op0=ALU.add)
                    sq = rot1.tile([128, T], BF16, tag="sq", bufs=3,
                                   name=f"sq{mt}")
                    nc.scalar.activation(out=sq[:], in_=qk_f[:], func=AF.Square)
                    pn = ps1.tile([128, T], F32, tag="nrm", bufs=2, name=f"pn{mt}")
                    ones = onesq_sb[:, mt, :] if mt < 8 else onesk_sb[:]
                    nc.tensor.matmul(pn[:], ones, sq[:], start=True, stop=True)
                    rrec = rot1.tile([128, T], F32, tag="rrec", name=f"rrec{mt}")
                    nc.scalar.activation(out=rrec[:], in_=pn[:],
                                         func=AF.Abs_reciprocal_sqrt)
                    return qk_f, rrec

                # ---- phase 3: v (token-major) -> bounce -> AG-v ----
                wv_sb = pa.tile([128, CT, C], BF16)
                nc.sync.dma_start(
                    out=wv_sb[:], in_=wv_d[:].rearrange("(k p) m -> p k m", p=128)
                )
                for t in range(TT):
                    v_sb = rot1.tile([128, C], BF16, tag="vsb", bufs=1, name=f"vsb{t}")
                    rstd_c = stat_cols[:, t:t + 1]
                    for n2 in range(2):
                        pv = ps1.tile([128, 512], F32, tag="mm", bufs=3,
                                      name=f"pv{t}{n2}")
                        for kc in range(CT):
                            nc.tensor.matmul(
                                pv[:],
                                xTb[kc][:, 128 * t:128 * (t + 1)],
                                wv_sb[:, kc, 512 * n2:512 * (n2 + 1)],
                                start=(kc == 0), stop=False,
                            )
                        nc.tensor.matmul(
                            pv[:],
                            negmu1[0:1, 128 * t:128 * (t + 1)],
                            cv_row[0:1, 512 * n2:512 * (n2 + 1)],
                            start=False, stop=True,
                        )
                        # v = rstd*(raw - mu*csumv) + vb
                        nc.vector.scalar_tensor_tensor(
                            out=v_sb[:, 512 * n2:512 * (n2 + 1)], in0=pv[:],
                            scalar=rstd_c,
                            in1=vb_bc[:, 512 * n2:512 * (n2 + 1)],
                            op0=ALU.mult, op1=ALU.add,
                        )
                    nc.gpsimd.dma_start(
                        out=bounce_v[256 * t:256 * (t + 1), :].rearrange(
                            "(p a) f -> p (a f)", p=128
                        ),
                        in_=v_sb[:],
                    )
                nc.gpsimd.collective_compute(
                    "AllGather", ALU.bypass,
                    replica_groups=[[0, 1, 2, 3], [4, 5, 6, 7]],
                    ins=[bounce_v.opt()], outs=[ag_v.opt()],
                )

                # ---- phase 2k: k rows -> bounce -> AG-k ----
                for mt in range(8, 16):
                    qk_f, rrec = qk_tile(mt)
                    i = mt - 8
                    ktn = rot1.tile([128, T], BF16, tag="ktn", bufs=2, name=f"ktn{i}")
                    nc.vector.tensor_mul(out=ktn[:], in0=qk_f[:], in1=rrec[:])
                    nc.gpsimd.dma_start(
                        out=bounce_k[128 * i:128 * (i + 1), :], in_=ktn[:]
                    )
                nc.gpsimd.collective_compute(
                    "AllGather", ALU.bypass,
                    replica_groups=[[0, 1, 2, 3], [4, 5, 6, 7]],
                    ins=[bounce_k.opt()], outs=[ag_k.opt()],
                )

                # prefetches (all data-ready at issue): out-proj weights,
                # residual xTf, first fc weights
                nc.sync.dma_start(
                    out=wo_sb[:], in_=wo_d[:].rearrange("(k p) m -> p k m", p=128)
                )
                for ct in range(CT):
                    nc.sync.dma_start(out=xTf[ct][:],
                                      in_=xtf_d[128 * ct:128 * (ct + 1), :])
                for mt in range(N_FC_PRE):
                    nc.sync.dma_start(out=wfc_pre[mt][:], in_=wfc_d[mt])

                # ---- phase 2q: q rows ----
                for mt in range(8):
                    qk_f, rrec = qk_tile(mt)
                    nc.vector.tensor_mul(out=qTn[mt][:], in0=qk_f[:], in1=rrec[:])

            # ---- phase 4: attention ----
            with (
                tc.tile_pool(name="pb", bufs=1) as pb,
                tc.tile_pool(name="rot2", bufs=2) as rot2,
                tc.tile_pool(name="ps2", bufs=1, space="PSUM") as ps2,
            ):
                for r in range(RANKS):
                    nc.vector.memset(v_ag[r][:], 1.0)
                    for sub in range(4):
                        vsrc = ag_v[
                            1024 * r + 256 * sub:1024 * r + 256 * (sub + 1), :
                        ].rearrange("(p a) f -> p (a f)", p=128).rearrange(
                            "p (h d) -> p h d", d=HD
                        )
                        nc.gpsimd.dma_start(out=v_ag[r][:, sub, :, 0:HD], in_=vsrc)
                ag_k_r = ag_k[:].rearrange("(r m p) f -> p r m f", r=RANKS, p=128)
                for hp in range(HP):
                    kpair = pb.tile([128, RANKS, T], BF16, tag="kpair", bufs=3,
                                    name=f"kpair{hp}")
                    nc.gpsimd.dma_start(out=kpair[:], in_=ag_k_r[:, :, hp, :])
                    po = [
                        ps2.tile([HD + 1, T], F32, tag=f"o{h2}", bufs=2,
                                 name=f"po{h2}")
                        for h2 in range(2)
                    ]
                    for km in range(KM):
                        r, sub = km // 4, km % 4
                        psS = ps2.tile([128, 2, T], F32, tag="s", bufs=2,
                                       name=f"psS{km}")
                        for h2 in range(2):
                            nc.tensor.matmul(
                                psS[:, h2, :],
                                kpair[64 * h2:64 * (h2 + 1), r,
                                      128 * sub:128 * (sub + 1)],
                                qTn[hp][64 * h2:64 * (h2 + 1), :],
                                start=True, stop=True,
                            )
                        pT = rot2.tile([128, 2, T], BF16, tag="pT", bufs=4,
                                       name=f"pT{km}")
                        nc.scalar.activation(out=pT[:], in_=psS[:], func=AF.Exp)
                        for h2 in range(2):
                            h = 2 * hp + h2
                            nc.tensor.matmul(
                                po[h2][:],
                                v_ag[r][:, sub, h, :],
                                pT[:, h2, :],
                                start=(km == 0), stop=(km == KM - 1),
                            )
                    for h2 in range(2):
                        h = 2 * hp + h2
                        # 1/rs = (1/sqrt(rs))^2; head_scale folded as sqrt(hs)
                        rs = rot2.tile([1, T], BF16, tag="rs")
                        nc.scalar.activation(out=rs[:], in_=po[h2][HD:HD + 1, :],
                                             func=AF.Abs_reciprocal_sqrt)
                        pr = ps2.tile([128, 2, T], F32, tag="s", bufs=2,
                                      name=f"pr{h2}")
                        nc.tensor.matmul(
                            pr[0:HD, 0, :],
                            invhs_sb[0:1, HD * h:HD * (h + 1)], rs[:],
                            start=True, stop=True,
                        )
                        rep2 = rot2.tile([HD, T], F32, tag="orec")
                        nc.scalar.activation(out=rep2[:], in_=pr[0:HD, 0, :],
                                             func=AF.Square)
                        nc.vector.tensor_mul(
                            out=OT[hp][64 * h2:64 * (h2 + 1), :],
                            in0=po[h2][0:HD, :],
                            in1=rep2[:],
                        )

            # ---- phase 5: out projection (transposed out) + ln_attn + ln2 ----
            with (
                tc.tile_pool(name="rot3", bufs=2) as rot3,
                tc.tile_pool(name="ps3", bufs=1, space="PSUM") as ps3,
            ):
                yT = [rot3.tile([128, T], F32, tag="yT", bufs=8, name=f"yT{m}")
                      for m in range(CT)]
                for m in range(CT):
                    py = ps3.tile([128, T], F32, tag="y", bufs=3, name=f"py{m}")
                    for kc in range(CT):
                        nc.tensor.matmul(
                            py[:],
                            wo_sb[:, kc, 128 * m:128 * (m + 1)],
                            OT[kc][:],
                            start=(kc == 0), stop=(kc == CT - 1),
                        )
                    nc.vector.tensor_scalar(out=yT[m][:], in0=py[:],
                                            scalar1=outb_sb[:, m:m + 1],
                                            scalar2=None, op0=ALU.add)
                a2_rep, c2_rep, _, _, _ = ln_stats_T(yT, rot3, ps3)
                # uT = xT + (yT - mu)*rstd*g' + b'   (g', b' have ls1 folded)
                for ct in range(CT):
                    t1 = rot3.tile([128, T], F32, tag="ut1", name=f"ut1_{ct}")
                    nc.vector.tensor_mul(out=t1[:], in0=yT[ct][:], in1=a2_rep[:])
                    t2 = rot3.tile([128, T], F32, tag="ut2", name=f"ut2_{ct}")
                    nc.vector.tensor_sub(out=t2[:], in0=t1[:], in1=c2_rep[:])
                    t3 = rot3.tile([128, T], F32, tag="ut3", name=f"ut3_{ct}")
                    nc.vector.tensor_scalar(out=t3[:], in0=t2[:],
                                            scalar1=ga_sb[:, ct:ct + 1],
                                            scalar2=ba_sb[:, ct:ct + 1],
                                            op0=ALU.mult, op1=ALU.add)
                    nc.vector.tensor_add(out=uT[ct][:], in0=t3[:], in1=xTf[ct][:])
                a3_rep, c3_rep, _, _, _ = ln_stats_T(uT, rot3, ps3)
                for ct in range(CT):
                    # h1T = (uT)*rstd - mu*rstd  (ln2 affine folded into fc)
                    t4 = rot3.tile([128, T], F32, tag="ut4", name=f"ut4_{ct}")
                    nc.vector.tensor_mul(out=t4[:], in0=uT[ct][:], in1=a3_rep[:])
                    nc.vector.tensor_sub(out=h1T[ct][:], in0=t4[:], in1=c3_rep[:])

            # ---- phase 7: fc + gelu;  phase 8: proj (transposed) + residual ----
            with (
                tc.tile_pool(name="pd", bufs=1) as pd,
                tc.tile_pool(name="rot4", bufs=2) as rot4,
                tc.tile_pool(name="ps4", bufs=1, space="PSUM") as ps4,
            ):
                gT = [pd.tile([128, T], BF16, name=f"gT{m}") for m in range(MT_FC)]
                for mt in range(MT_FC):
                    if mt < N_FC_PRE:
                        wfc_t = wfc_pre[mt]
                    else:
                        wfc_t = rot4.tile([128, CT, 128], BF16, tag="wfc", bufs=6,
                                          name=f"wfc{mt}")
                        nc.sync.dma_start(out=wfc_t[:], in_=wfc_d[mt])
                    pfc = ps4.tile([128, T], F32, tag="fc", bufs=3, name=f"pfc{mt}")
                    for kc in range(CT):
                        nc.tensor.matmul(
                            pfc[:], wfc_t[:, kc, :], h1T[kc][:],
                            start=(kc == 0), stop=(kc == CT - 1),
                        )
                    nc.scalar.activation(out=gT[mt][:], in_=pfc[:], func=AF.Gelu,
                                         bias=fcb_sb[:, mt:mt + 1])

                for half in range(2):
                    ppj = [
                        ps4.tile([128, T], F32, tag=f"pj{i}", bufs=1,
                                 name=f"ppj{i}")
                        for i in range(4)
                    ]
                    for mt in range(MT_FC):
                        wpj_t = rot4.tile([128, 4, 128], BF16, tag="wpj", bufs=8,
                                          name=f"wpj{half}_{mt}")
                        nc.sync.dma_start(
                            out=wpj_t[:],
                            in_=wpj_d[128 * mt:128 * (mt + 1),
                                      512 * half:512 * (half + 1)].rearrange(
                                "p (i m) -> p i m", m=128
                            ),
                        )
                        for i in range(4):
                            nc.tensor.matmul(
                                ppj[i][:],
                                wpj_t[:, i, :],
                                gT[mt][:],
                                start=(mt == 0), stop=(mt == MT_FC - 1),
                            )
                    for i in range(4):
                        ct = 4 * half + i
                        o1 = rot4.tile([128, T], F32, tag="ofin",
                                       name=f"of{half}_{i}")
                        nc.vector.tensor_scalar(out=o1[:], in0=ppj[i][:],
                                                scalar1=pjb_sb[:, ct:ct + 1],
                                                scalar2=None, op0=ALU.add)
                        nc.vector.tensor_add(out=o1[:], in0=o1[:], in1=uT[ct][:])
                        nc.gpsimd.dma_start(
                            out=out_d[128 * ct:128 * (ct + 1), :], in_=o1[:]
                        )

    nc.compile()
    return nc


def _host_prep(inp):
    f32 = np.float32
    ln1_g = np.asarray(inp["ln1_g"], f32)
    ln1_b = np.asarray(inp["ln1_b"], f32)
    ln2_g = np.asarray(inp["ln2_g"], f32)
    ln2_b = np.asarray(inp["ln2_b"], f32)
    in_w = np.asarray(inp["in_proj_w"], f32)
    in_b = np.asarray(inp["in_proj_b"], f32)
    fc_w = np.asarray(inp["fc_w"], f32)
    proj_w = np.asarray(inp["proj_w"], f32)
    ls1 = np.asarray(inp["ls1"], f32)
    ls2 = np.asarray(inp["ls2"], f32)

    w_qk = in_w[:2 * C]
    w_v = in_w[2 * C:]
    p = {}
    wqkT = np.ascontiguousarray((w_qk * ln1_g[None, :]).T).astype(BF_NP)
    wvT = np.ascontiguousarray((w_v * ln1_g[None, :]).T).astype(BF_NP)
    p["wqkT"] = np.ascontiguousarray(
        wqkT.reshape(CT, 128, 16, 128).transpose(2, 1, 0, 3)
    )
    p["wvT"] = wvT
    # column sums of the bf16 weights, as the device matmul sees them
    csum_qk = wqkT.astype(f32).sum(0)
    csum_v = wvT.astype(f32).sum(0)
    p["csumqk"] = csum_qk.reshape(1, 2 * C).astype(f32)
    p["csumv"] = csum_v.reshape(1, C).astype(f32)
    qkb = in_b[:2 * C] + ln1_b @ w_qk.T
    p["qkb"] = np.ascontiguousarray(qkb.reshape(16, 128).T).astype(f32)
    p["vb"] = (in_b[2 * C:] + ln1_b @ w_v.T).reshape(1, C).astype(f32)
    p["woT"] = np.ascontiguousarray(np.asarray(inp["out_w"], f32).T).astype(BF_NP)
    p["outb_s"] = np.ascontiguousarray(
        np.asarray(inp["out_b"], f32).reshape(8, 128).T
    )
    wfcT = np.ascontiguousarray((fc_w * ln2_g[None, :]).T).astype(BF_NP)
    p["wfcT"] = np.ascontiguousarray(
        wfcT.reshape(CT, 128, MT_FC, 128).transpose(2, 1, 0, 3)
    )
    fcb = np.asarray(inp["fc_b"], f32) + ln2_b @ fc_w.T
    p["fcb"] = np.ascontiguousarray(fcb.reshape(32, 128).T).astype(f32)
    p["wprojT"] = np.ascontiguousarray((proj_w * ls2[:, None]).T).astype(BF_NP)
    p["projb_s"] = np.ascontiguousarray(
        (ls2 * np.asarray(inp["proj_b"], f32)).reshape(8, 128).T
    )
    p["gattn_s"] = np.ascontiguousarray(
        (ls1 * np.asarray(inp["ln_attn_g"], f32)).reshape(8, 128).T
    )
    p["battn_s"] = np.ascontiguousarray(
        (ls1 * np.asarray(inp["ln_attn_b"], f32)).reshape(8, 128).T
    )

    lsc = np.exp(np.minimum(np.asarray(inp["logit_scale"], f32).reshape(H),
                            np.log(100.0)))
    onesq = np.zeros((128, 8, 128), f32)
    for mt in range(8):
        for blk in range(2):
            h = 2 * mt + blk
            onesq[64 * blk:64 * (blk + 1), mt,
                  64 * blk:64 * (blk + 1)] = 1.0 / lsc[h] ** 2
    p["onesq"] = np.ascontiguousarray(onesq.reshape(128, 1024)).astype(BF_NP)
    onesk = np.zeros((128, 128), f32)
    onesk[:64, :64] = 1.0
    onesk[64:, 64:] = 1.0
    p["onesk"] = onesk.astype(BF_NP)
    hs = np.asarray(inp["head_scale"], f32)
    invhs = np.zeros((1, C), f32)
    for h in range(H):
        invhs[0, HD * h:HD * (h + 1)] = np.sqrt(hs[h])
    p["invhs"] = invhs.astype(BF_NP)
    return p


def kernel(**inputs) -> np.ndarray:
    global _NC_CACHE, LAST_EXEC_NS, LAST_RESULTS
    if _NC_CACHE is None:
        _NC_CACHE = _build()
    nc = _NC_CACHE

    p = _host_prep(inputs)
    x = np.asarray(inputs["x"], np.float32)

    in_maps = []
    for c in range(N_CORES):
        b, r = c // RANKS, c % RANKS
        m = dict(p)
        xs = np.ascontiguousarray(x[b, T * r:T * (r + 1), :].T)   # [C, T]
        m["xTf"] = xs
        m["xTb"] = xs.astype(BF_NP)
        in_maps.append(m)

    kwargs = {}
    if TRACE:
        import os
        os.makedirs(TRACE_DIR, exist_ok=True)
        kwargs = dict(trace=True, tmpdir=TRACE_DIR)
    res = run_bass_kernel_spmd(nc, in_maps, list(range(N_CORES)), **kwargs)
    LAST_EXEC_NS = res.exec_time_ns
    LAST_RESULTS = res
    out = np.zeros((B, L, C), np.float32)
    for c in range(N_CORES):
        b, r = c // RANKS, c % RANKS
        out[b, T * r:T * (r + 1), :] = res.results[c]["outT"].T
    return out
